# revision 12
# baseline (speedup 1.0000x reference)
"""Trainium2 Bass kernel for nn_Attention_89902255440825.

Single-layer attention block: QKV proj + per-head RMS("mult" variant) +
RoPE + GQA causal attention with softmax(scores * sqrt(HD)) + O proj.

Sharding (8 NeuronCores, tensor-parallel over heads):
  core c: q heads {2c, 2c+1}  (wq cols 256c:256c+256)
          kv head c//2        (wk/wv cols 128*(c//2):...)
          wo rows 256c:256c+256  -> partial [S,H] outputs, summed on host.

Precision strategy (measured on HW):
  - q/k path (projections + scores) uses fp16 hi/lo split: 3 fp16 matmuls
    give ~fp32 accuracy at 3 cyc/row (native fp32 is 4 cyc/row).
    Required because softmax here is multiplied by sqrt(HD): logit std ~95,
    near-argmax attention, so logits need ~1e-5 relative accuracy.
  - v / PV / O-proj use single-pass fp16 (error ~1e-3, benign).

Layouts on device (per core):
  xT (hi/lo fp16)      [H=2048, S=2048]   host-pretransposed
  qT/kT feature-major  [d=128, m=2048]    d on partitions (scores lhsT/rhs)
  v    token-major     [m, d] tiles       (PV rhs)
  attn_outT via PE transpose -> O proj lhsT
"""
import numpy as np
from contextlib import ExitStack

import concourse.bass as bass
import concourse.tile as tile
from concourse import bacc, mybir, bass_utils
from concourse.masks import make_identity

S = 2048
H = 2048
HD = 128
NH = 16
NKV = 4
NCORES = 8
HPC = NH // NCORES          # q heads per core = 2
FQ = HPC * HD               # q features per core = 256
EPS = 1e-6
SM_SCALE = float(np.sqrt(HD))   # reference: softmax(scores / HD**-0.5)
NEG = -1e30
F32 = mybir.dt.float32
F16 = mybir.dt.float16
AX = mybir.AxisListType.X
OP = mybir.AluOpType
ACTF = mybir.ActivationFunctionType

_prog_cache = {}


def _build(is_causal: bool):
    nc = bacc.Bacc("TRN2", target_bir_lowering=False, debug=False,
                   num_devices=NCORES)

    def din(name, shape, dt=F16):
        return nc.dram_tensor(name, shape, dt, kind="ExternalInput").ap()

    xth_d = din("xth", [H, S])
    xtl_d = din("xtl", [H, S])
    wqh_d = din("wqh", [H, FQ])
    wql_d = din("wql", [H, FQ])
    wkh_d = din("wkh", [H, HD])
    wkl_d = din("wkl", [H, HD])
    wvh_d = din("wvh", [H, HD])
    woh_d = din("woh", [FQ, H])
    cost_d = din("cost", [HD, S], F32)
    sint_d = din("sint", [HD, S], F32)
    bqt_d = din("bqt", [HD, HPC], F32)
    bkt_d = din("bkt", [HD, 1], F32)
    bvt_d = din("bvt", [HD, 1], F32)
    qn_d = din("qn", [HD, 1], F32)
    kn_d = din("kn", [HD, 1], F32)
    if not is_causal:
        mask_d = din("maskadd", [S, S], F32)
    out_d = nc.dram_tensor("out", [S, H], F32, kind="ExternalOutput").ap()

    NKB = H // 128            # 16 contraction k-blocks
    NMB = S // 128            # 16 token blocks
    NCH = S // 512            # 4 512-chunks

    with tile.TileContext(nc) as tc, ExitStack() as ctx:
        const = ctx.enter_context(tc.tile_pool(name="const", bufs=1))
        wpool = ctx.enter_context(tc.tile_pool(name="wpool", bufs=1))
        big = ctx.enter_context(tc.tile_pool(name="big", bufs=1))
        xpool = ctx.enter_context(tc.tile_pool(name="xpool", bufs=3))
        btmp = ctx.enter_context(tc.tile_pool(name="btmp", bufs=2))
        cpool = ctx.enter_context(tc.tile_pool(name="cpool", bufs=2))
        dpool = ctx.enter_context(tc.tile_pool(name="dpool", bufs=3))
        psum = ctx.enter_context(tc.tile_pool(name="psum", bufs=1, space="PSUM"))

        # ---- constants ----
        ident16 = const.tile([128, 128], F16)
        make_identity(nc, ident16[:])
        ones_col = const.tile([128, 1], F32)
        nc.vector.memset(ones_col[:], 1.0)
        bcq = const.tile([1, 128], F32)          # broadcast row incl. sm scale
        nc.vector.memset(bcq[:], SM_SCALE)
        bck = const.tile([1, 128], F32)
        nc.vector.memset(bck[:], 1.0)
        eps_t = const.tile([1, 1], F32)
        nc.vector.memset(eps_t[:], EPS)
        if is_causal:
            cmask = const.tile([128, 4, 512], F32)
            for r in range(4):
                nc.vector.memset(cmask[:, r, :], 0.0)
                nc.gpsimd.affine_select(
                    out=cmask[:, r, :], in_=cmask[:, r, :],
                    compare_op=OP.is_ge, fill=NEG,
                    base=128 * r, channel_multiplier=1, pattern=[[-1, 512]],
                )

        # ---- weights / small inputs to SBUF ----
        def wtiles(dram, f, nm):
            t = wpool.tile([128, NKB, f], F16, name=nm, tag=nm)
            nc.sync.dma_start(t[:], dram.rearrange("(t p) f -> p t f", p=128))
            return t

        wqh_sb = wtiles(wqh_d, FQ, "wqh_sb")
        wql_sb = wtiles(wql_d, FQ, "wql_sb")
        wkh_sb = wtiles(wkh_d, HD, "wkh_sb")
        wkl_sb = wtiles(wkl_d, HD, "wkl_sb")
        wvh_sb = wtiles(wvh_d, HD, "wvh_sb")
        woh_sb = wpool.tile([128, HPC, H], F16)
        nc.sync.dma_start(woh_sb[:], woh_d.rearrange("(t p) f -> p t f", p=128))
        cost_sb = wpool.tile([HD, S], F32)
        nc.sync.dma_start(cost_sb[:], cost_d)
        sint_sb = wpool.tile([HD, S], F32)
        nc.sync.dma_start(sint_sb[:], sint_d)
        bqt_sb = wpool.tile([HD, HPC], F32)
        nc.sync.dma_start(bqt_sb[:], bqt_d)
        bkt_sb = wpool.tile([HD, 1], F32)
        nc.sync.dma_start(bkt_sb[:], bkt_d)
        bvt_sb = wpool.tile([HD, 1], F32)
        nc.sync.dma_start(bvt_sb[:], bvt_d)
        qn_sb = wpool.tile([HD, 1], F32)
        nc.sync.dma_start(qn_sb[:], qn_d)
        kn_sb = wpool.tile([HD, 1], F32)
        nc.sync.dma_start(kn_sb[:], kn_d)

        # ---- persistent activations ----
        qt_f32 = big.tile([128, HPC, S], F32)     # q feature-major, pre-norm
        kt_f32 = big.tile([128, S], F32)
        vt16 = big.tile([128, S], F16)            # v feature-major fp16
        v_sb = big.tile([128, NMB, 128], F16)     # v token-major fp16
        qh16 = big.tile([128, HPC, S], F16)       # roped+scaled q hi/lo
        ql16 = big.tile([128, HPC, S], F16)
        kh16 = big.tile([128, S], F16)
        kl16 = big.tile([128, S], F16)

        # ================= Phase A: QKV projections =================
        # feature-major: psum[f 128, m 512] += w[kb,f].T @ xT[kb, mquarter]
        for mq_ in range(4):
            ms = slice(mq_ * 512, (mq_ + 1) * 512)
            pq = [psum.tile([128, 512], F32, tag="t512", bufs=4, name=f"pq{fb}_{mq_}")
                  for fb in range(HPC)]
            pk = psum.tile([128, 512], F32, tag="t512", bufs=4)
            pv = psum.tile([128, 512], F32, tag="t512", bufs=4)
            for kb in range(NKB):
                xh_t = xpool.tile([128, 512], F16, tag="xh")
                nc.sync.dma_start(xh_t[:], xth_d[kb * 128:(kb + 1) * 128, ms])
                xl_t = xpool.tile([128, 512], F16, tag="xl")
                nc.sync.dma_start(xl_t[:], xtl_d[kb * 128:(kb + 1) * 128, ms])
                st = kb == 0
                sp = kb == NKB - 1
                for fb in range(HPC):
                    fsl = slice(fb * 128, (fb + 1) * 128)
                    nc.tensor.matmul(pq[fb][:], wqh_sb[:, kb, fsl], xh_t[:],
                                     start=st, stop=False)
                    nc.tensor.matmul(pq[fb][:], wqh_sb[:, kb, fsl], xl_t[:],
                                     start=False, stop=False)
                    nc.tensor.matmul(pq[fb][:], wql_sb[:, kb, fsl], xh_t[:],
                                     start=False, stop=sp)
                nc.tensor.matmul(pk[:], wkh_sb[:, kb, :], xh_t[:],
                                 start=st, stop=False)
                nc.tensor.matmul(pk[:], wkh_sb[:, kb, :], xl_t[:],
                                 start=False, stop=False)
                nc.tensor.matmul(pk[:], wkl_sb[:, kb, :], xh_t[:],
                                 start=False, stop=sp)
                nc.tensor.matmul(pv[:], wvh_sb[:, kb, :], xh_t[:],
                                 start=st, stop=sp)
            for fb in range(HPC):
                nc.vector.tensor_scalar_add(qt_f32[:, fb, ms], pq[fb][:],
                                            bqt_sb[:, fb:fb + 1])
            nc.vector.tensor_scalar_add(kt_f32[:, ms], pk[:], bkt_sb[:])
            nc.vector.tensor_scalar_add(vt16[:, ms], pv[:], bvt_sb[:])

        # v: feature-major -> token-major via PE transpose
        for mb in range(NMB):
            pvt = psum.tile([128, 128], F16, tag="t128", bufs=4)
            nc.tensor.transpose(pvt[:], vt16[:, mb * 128:(mb + 1) * 128],
                                ident16[:])
            nc.vector.tensor_copy(v_sb[:, mb], pvt[:])

        # ================= Phase B: RMS + RoPE + split =================
        # rms scale s = sqrt(mean(q^2)+eps) per token (partition-dim reduce
        # via PE ones-matmul); broadcast back via K=1 PE matmul.
        specs = [
            (qt_f32[:, 0], qh16[:, 0], ql16[:, 0], bcq, qn_sb),
            (qt_f32[:, 1], qh16[:, 1], ql16[:, 1], bcq, qn_sb),
            (kt_f32[:], kh16[:], kl16[:], bck, kn_sb),
        ]
        for src, dsth, dstl, bcast, nw in specs:
            for ch in range(NCH):
                sl = slice(ch * 512, (ch + 1) * 512)
                sq = btmp.tile([128, 512], F32, tag="sq")
                nc.scalar.activation(sq[:], src[:, sl], ACTF.Square)
                pss = psum.tile([1, 512], F32, tag="t512", bufs=4)
                nc.tensor.matmul(pss[:], ones_col[:], sq[:],
                                 start=True, stop=True)
                ssb = btmp.tile([1, 512], F32, tag="ssb")
                nc.scalar.activation(ssb[:], pss[:], ACTF.Sqrt,
                                     bias=eps_t[:], scale=1.0 / HD)
                pbc = psum.tile([128, 512], F32, tag="t512", bufs=4)
                nc.tensor.matmul(pbc[:], bcast[:], ssb[:],
                                 start=True, stop=True)
                t1 = btmp.tile([128, 512], F32, tag="t1")
                nc.vector.tensor_mul(t1[:], src[:, sl], pbc[:])
                nc.vector.tensor_scalar_mul(t1[:], t1[:], nw[:])
                rot = btmp.tile([128, 512], F32, tag="rot")
                nc.vector.tensor_scalar_mul(rot[0:64, :], t1[64:128, :], -1.0)
                nc.vector.tensor_copy(rot[64:128, :], t1[0:64, :])
                qr = btmp.tile([128, 512], F32, tag="qr")
                nc.vector.tensor_mul(qr[:], t1[:], cost_sb[:, sl])
                nc.vector.tensor_mul(rot[:], rot[:], sint_sb[:, sl])
                nc.vector.tensor_add(qr[:], qr[:], rot[:])
                nc.vector.tensor_copy(dsth[:, sl], qr[:])
                nc.vector.tensor_sub(dstl[:, sl], qr[:], dsth[:, sl])

        # ================= Phase C/D: attention + O proj =================
        for i in range(NMB):
            nchunks = (i // 4 + 1) if is_causal else NCH
            attn16 = cpool.tile([128, HPC, 128], F16, tag="attn16")
            s_sbs, negms, lpartss, out_pss = [], [], [], []
            # pass 1 (both heads): scores (3x fp16 split matmuls), row maxes
            for h in range(HPC):
                qh_blk = qh16[:, h, i * 128:(i + 1) * 128]
                ql_blk = ql16[:, h, i * 128:(i + 1) * 128]
                s_sb = cpool.tile([128, NCH, 512], F32, tag="s_sb",
                                  name=f"s_sb_{i}_{h}")
                mparts = cpool.tile([128, NCH], F32, tag="mparts",
                                    name=f"mparts_{i}_{h}")
                for ncj in range(nchunks):
                    ks = slice(ncj * 512, (ncj + 1) * 512)
                    ps_s = psum.tile([128, 512], F32, tag="t512", bufs=4,
                                     name=f"ps_s_{i}_{h}_{ncj}")
                    nc.tensor.matmul(ps_s[:], qh_blk, kh16[:, ks],
                                     start=True, stop=False)
                    nc.tensor.matmul(ps_s[:], qh_blk, kl16[:, ks],
                                     start=False, stop=False)
                    nc.tensor.matmul(ps_s[:], ql_blk, kh16[:, ks],
                                     start=False, stop=True)
                    if is_causal and ncj == i // 4:
                        nc.vector.tensor_add(s_sb[:, ncj, :], ps_s[:],
                                             cmask[:, i % 4, :])
                    elif not is_causal:
                        mload = cpool.tile([128, 512], F32, tag="mload",
                                           bufs=3, name=f"mload_{i}_{h}_{ncj}")
                        nc.sync.dma_start(
                            mload[:], mask_d[i * 128:(i + 1) * 128, ks])
                        nc.vector.tensor_add(s_sb[:, ncj, :], ps_s[:],
                                             mload[:])
                    else:
                        nc.scalar.copy(s_sb[:, ncj, :], ps_s[:])
                    masked = (is_causal and ncj == i // 4) or not is_causal
                    nc.vector.reduce_max(mparts[:, ncj:ncj + 1],
                                         s_sb[:, ncj, :] if masked else ps_s[:],
                                         axis=AX)
                negm = cpool.tile([128, 1], F32, tag="negm",
                                  name=f"negm_{i}_{h}")
                nc.vector.reduce_max(negm[:], mparts[:, 0:nchunks], axis=AX,
                                     negate=True)
                s_sbs.append(s_sb)
                negms.append(negm)
            # pass 2 (both heads): exp (fp16) -> PE transpose -> PV accumulate
            for h in range(HPC):
                s_sb, negm = s_sbs[h], negms[h]
                lparts = cpool.tile([128, NCH], F32, tag="lparts",
                                    name=f"lparts_{i}_{h}")
                out_ps = psum.tile([128, 128], F32, tag="t128", bufs=4,
                                   name=f"out_ps_{i}_{h}")
                last_nkb = i if is_causal else nchunks * 4 - 1
                for ncj in range(nchunks):
                    p16 = cpool.tile([128, 512], F16, tag="p16",
                                     name=f"p16_{i}_{h}_{ncj}")
                    nc.scalar.activation(p16[:], s_sb[:, ncj, :], ACTF.Exp,
                                         bias=negm[:], scale=1.0,
                                         accum_out=lparts[:, ncj:ncj + 1])
                    nb = min(4, last_nkb + 1 - ncj * 4)
                    ps_t4 = psum.tile([128, 4, 128], F16, tag="t128", bufs=4,
                                      name=f"ps_t4_{i}_{h}_{ncj}")
                    for b in range(nb):
                        nc.tensor.transpose(
                            ps_t4[:, b, :], p16[:, b * 128:(b + 1) * 128],
                            ident16[:])
                    pt_sb = cpool.tile([128, 4, 128], F16, tag="pt_sb",
                                       bufs=4, name=f"pt_sb_{i}_{h}_{ncj}")
                    if ncj % 2 == 0:
                        nc.vector.tensor_copy(pt_sb[:, 0:nb, :],
                                              ps_t4[:, 0:nb, :])
                    else:
                        nc.scalar.copy(pt_sb[:, 0:nb, :], ps_t4[:, 0:nb, :])
                    for b in range(nb):
                        nkb = ncj * 4 + b
                        nc.tensor.matmul(out_ps[:], pt_sb[:, b, :],
                                         v_sb[:, nkb],
                                         start=(nkb == 0),
                                         stop=(nkb == last_nkb))
                lpartss.append(lparts)
                out_pss.append(out_ps)
            for h in range(HPC):
                lsum = cpool.tile([128, 1], F32, tag="lsum",
                                  name=f"lsum_{i}_{h}")
                nc.vector.reduce_sum(lsum[:], lpartss[h][:, 0:nchunks],
                                     axis=AX)
                linv = cpool.tile([128, 1], F32, tag="linv",
                                  name=f"linv_{i}_{h}")
                nc.vector.reciprocal(linv[:], lsum[:])
                at = cpool.tile([128, 128], F16, tag="at", name=f"at_{i}_{h}")
                nc.vector.tensor_scalar_mul(at[:], out_pss[h][:], linv[:])
                pat = psum.tile([128, 128], F16, tag="t128", bufs=4,
                                name=f"pat_{i}_{h}")
                nc.tensor.transpose(pat[:], at[:], ident16[:])
                nc.vector.tensor_copy(attn16[:, h], pat[:])
            # O proj partial: out[m, n] += attnT[f, m].T @ wo[f, n]
            for nh_ in range(4):
                ns = slice(nh_ * 512, (nh_ + 1) * 512)
                po = psum.tile([128, 512], F32, tag="t512", bufs=4,
                               name=f"po_{i}_{nh_}")
                nc.tensor.matmul(po[:], attn16[:, 0], woh_sb[:, 0, ns],
                                 start=True, stop=False)
                nc.tensor.matmul(po[:], attn16[:, 1], woh_sb[:, 1, ns],
                                 start=False, stop=True)
                ob = dpool.tile([128, 512], F32, tag="ob",
                                name=f"ob_{i}_{nh_}")
                if nh_ % 2 == 0:
                    nc.vector.tensor_copy(ob[:], po[:])
                else:
                    nc.scalar.copy(ob[:], po[:])
                nc.sync.dma_start(out_d[i * 128:(i + 1) * 128, ns], ob[:])

    nc.compile()
    return nc


def _split16(a):
    hi = a.astype(np.float16)
    lo = (a - hi.astype(np.float32)).astype(np.float16)
    return hi, lo


def kernel(**inputs):
    x = np.asarray(inputs["x"], np.float32)
    cos = np.asarray(inputs["cos"], np.float32)
    sin = np.asarray(inputs["sin"], np.float32)
    am = np.asarray(inputs["attention_mask"]).reshape(S, S).astype(bool)
    wq = np.asarray(inputs["wq"], np.float32)
    bq = np.asarray(inputs["bq"], np.float32)
    wk = np.asarray(inputs["wk"], np.float32)
    bk = np.asarray(inputs["bk"], np.float32)
    wv = np.asarray(inputs["wv"], np.float32)
    bv = np.asarray(inputs["bv"], np.float32)
    wo = np.asarray(inputs["wo"], np.float32)
    bo = np.asarray(inputs["bo"], np.float32)
    qn = np.asarray(inputs["q_norm_w"], np.float32)
    kn = np.asarray(inputs["k_norm_w"], np.float32)

    assert x.shape == (1, S, H)
    is_causal = bool(
        (am == np.triu(np.ones((S, S), dtype=bool), k=1)).all())

    key = is_causal
    if key not in _prog_cache:
        _prog_cache[key] = _build(is_causal)
    nc = _prog_cache[key]

    xT = np.ascontiguousarray(x[0].T)
    xth, xtl = _split16(xT)
    cosT = np.ascontiguousarray(cos.T)
    sinT = np.ascontiguousarray(sin.T)
    qn_c = np.ascontiguousarray(qn.reshape(HD, 1))
    kn_c = np.ascontiguousarray(kn.reshape(HD, 1))
    if not is_causal:
        maskadd = np.where(am, np.float32(NEG), np.float32(0.0))

    in_maps = []
    for c in range(NCORES):
        fq = slice(c * FQ, (c + 1) * FQ)
        g = c // 2
        fk = slice(g * HD, (g + 1) * HD)
        wqh, wql = _split16(wq[:, fq])
        wkh, wkl = _split16(wk[:, fk])
        m = dict(
            xth=xth, xtl=xtl,
            wqh=np.ascontiguousarray(wqh), wql=np.ascontiguousarray(wql),
            wkh=np.ascontiguousarray(wkh), wkl=np.ascontiguousarray(wkl),
            wvh=np.ascontiguousarray(wv[:, fk].astype(np.float16)),
            woh=np.ascontiguousarray(wo[fq, :].astype(np.float16)),
            cost=cosT, sint=sinT,
            bqt=np.ascontiguousarray(bq[fq].reshape(HPC, HD).T),
            bkt=np.ascontiguousarray(bk[fk].reshape(1, HD).T),
            bvt=np.ascontiguousarray(bv[fk].reshape(1, HD).T),
            qn=qn_c, kn=kn_c,
        )
        if not is_causal:
            m["maskadd"] = maskadd
        in_maps.append(m)

    res = bass_utils.run_bass_kernel_spmd(nc, in_maps,
                                          core_ids=list(range(NCORES)))
    acc = np.zeros((S, H), np.float64)
    for c in range(NCORES):
        acc += res.results[c]["out"]
    out = (acc + bo[None, :]).astype(np.float32)
    return out.reshape(1, S, H)


# revision 14
# speedup vs baseline: 1.0881x; 1.0881x over previous
"""Trainium2 Bass kernel for nn_Attention_89902255440825.

Single-layer attention block: QKV proj + per-head RMS("mult" variant) +
RoPE + GQA causal attention with softmax(scores * sqrt(HD)) + O proj.

Sharding (8 NeuronCores, tensor-parallel over heads):
  core c: q heads {2c, 2c+1}  (wq cols 256c:256c+256)
          kv head c//2        (wk/wv cols 128*(c//2):...)
          wo rows 256c:256c+256  -> partial [S,H] outputs, summed on host.

Precision strategy (measured on HW):
  - q/k path (projections + scores) uses fp16 hi/lo split: 3 fp16 matmuls
    give ~fp32 accuracy at 3 cyc/row (native fp32 is 4 cyc/row).
    Required because softmax here is multiplied by sqrt(HD): logit std ~95,
    near-argmax attention, so logits need ~1e-5 relative accuracy.
  - v / PV / O-proj use single-pass fp16 (error ~1e-3, benign).

Layouts on device (per core):
  xT (hi/lo fp16)      [H=2048, S=2048]   host-pretransposed
  qT/kT feature-major  [d=128, m=2048]    d on partitions (scores lhsT/rhs)
  v    token-major     [m, d] tiles       (PV rhs)
  attn_outT via PE transpose -> O proj lhsT
"""
import numpy as np
from contextlib import ExitStack

import concourse.bass as bass
import concourse.tile as tile
from concourse import bacc, mybir, bass_utils
from concourse.masks import make_identity

S = 2048
H = 2048
HD = 128
NH = 16
NKV = 4
NCORES = 8
HPC = NH // NCORES          # q heads per core = 2
FQ = HPC * HD               # q features per core = 256
EPS = 1e-6
SM_SCALE = float(np.sqrt(HD))   # reference: softmax(scores / HD**-0.5)
NEG = -1e30
F32 = mybir.dt.float32
F16 = mybir.dt.float16
AX = mybir.AxisListType.X
OP = mybir.AluOpType
ACTF = mybir.ActivationFunctionType

_prog_cache = {}


def _build(is_causal: bool):
    nc = bacc.Bacc("TRN2", target_bir_lowering=False, debug=False,
                   num_devices=NCORES)

    def din(name, shape, dt=F16):
        return nc.dram_tensor(name, shape, dt, kind="ExternalInput").ap()

    xth_d = din("xth", [H, S])
    xtl_d = din("xtl", [H, S])
    wqh_d = din("wqh", [H, FQ])
    wql_d = din("wql", [H, FQ])
    wkh_d = din("wkh", [H, HD])
    wkl_d = din("wkl", [H, HD])
    wvh_d = din("wvh", [H, HD])
    woh_d = din("woh", [FQ, H])
    cost_d = din("cost", [HD, S], F32)
    sint_d = din("sint", [HD, S], F32)
    bqt_d = din("bqt", [HD, HPC], F32)
    bkt_d = din("bkt", [HD, 1], F32)
    bvt_d = din("bvt", [HD, 1], F32)
    qn_d = din("qn", [HD, 1], F32)
    kn_d = din("kn", [HD, 1], F32)
    if not is_causal:
        mask_d = din("maskadd", [S, S], F32)
    out_d = nc.dram_tensor("out", [S, H], F32, kind="ExternalOutput").ap()

    NKB = H // 128            # 16 contraction k-blocks
    NMB = S // 128            # 16 token blocks
    NCH = S // 512            # 4 512-chunks

    with tile.TileContext(nc) as tc, ExitStack() as ctx:
        const = ctx.enter_context(tc.tile_pool(name="const", bufs=1))
        wpool = ctx.enter_context(tc.tile_pool(name="wpool", bufs=1))
        big = ctx.enter_context(tc.tile_pool(name="big", bufs=1))
        xpool = ctx.enter_context(tc.tile_pool(name="xpool", bufs=3))
        btmp = ctx.enter_context(tc.tile_pool(name="btmp", bufs=2))
        cpool = ctx.enter_context(tc.tile_pool(name="cpool", bufs=2))
        dpool = ctx.enter_context(tc.tile_pool(name="dpool", bufs=3))
        psum = ctx.enter_context(tc.tile_pool(name="psum", bufs=1, space="PSUM"))

        # ---- constants ----
        ident16 = const.tile([128, 128], F16)
        make_identity(nc, ident16[:])
        ones_col = const.tile([128, 1], F32)
        nc.vector.memset(ones_col[:], 1.0)
        bcq = const.tile([1, 128], F32)          # broadcast row incl. sm scale
        nc.vector.memset(bcq[:], SM_SCALE)
        bck = const.tile([1, 128], F32)
        nc.vector.memset(bck[:], 1.0)
        eps_t = const.tile([1, 1], F32)
        nc.vector.memset(eps_t[:], EPS)
        if is_causal:
            cmask = const.tile([128, 4, 512], F32)
            for r in range(4):
                nc.vector.memset(cmask[:, r, :], 0.0)
                nc.gpsimd.affine_select(
                    out=cmask[:, r, :], in_=cmask[:, r, :],
                    compare_op=OP.is_ge, fill=NEG,
                    base=128 * r, channel_multiplier=1, pattern=[[-1, 512]],
                )

        # ---- weights / small inputs to SBUF ----
        def wtiles(dram, f, nm):
            t = wpool.tile([128, NKB, f], F16, name=nm, tag=nm)
            nc.sync.dma_start(t[:], dram.rearrange("(t p) f -> p t f", p=128))
            return t

        wqh_sb = wtiles(wqh_d, FQ, "wqh_sb")
        wql_sb = wtiles(wql_d, FQ, "wql_sb")
        wkh_sb = wtiles(wkh_d, HD, "wkh_sb")
        wkl_sb = wtiles(wkl_d, HD, "wkl_sb")
        wvh_sb = wtiles(wvh_d, HD, "wvh_sb")
        woh_sb = wpool.tile([128, HPC, H], F16)
        nc.sync.dma_start(woh_sb[:], woh_d.rearrange("(t p) f -> p t f", p=128))
        cost_sb = wpool.tile([HD, S], F32)
        nc.sync.dma_start(cost_sb[:], cost_d)
        sint_sb = wpool.tile([HD, S], F32)
        nc.sync.dma_start(sint_sb[:], sint_d)
        bqt_sb = wpool.tile([HD, HPC], F32)
        nc.sync.dma_start(bqt_sb[:], bqt_d)
        bkt_sb = wpool.tile([HD, 1], F32)
        nc.sync.dma_start(bkt_sb[:], bkt_d)
        bvt_sb = wpool.tile([HD, 1], F32)
        nc.sync.dma_start(bvt_sb[:], bvt_d)
        qn_sb = wpool.tile([HD, 1], F32)
        nc.sync.dma_start(qn_sb[:], qn_d)
        kn_sb = wpool.tile([HD, 1], F32)
        nc.sync.dma_start(kn_sb[:], kn_d)

        # ---- persistent activations ----
        qt_f32 = big.tile([128, HPC, S], F32)     # q feature-major, pre-norm
        kt_f32 = big.tile([128, S], F32)
        vt16 = big.tile([128, S], F16)            # v feature-major fp16
        v_sb = big.tile([128, NMB, 128], F16)     # v token-major fp16
        qh16 = big.tile([128, HPC, S], F16)       # roped+scaled q hi/lo
        ql16 = big.tile([128, HPC, S], F16)
        kh16 = big.tile([128, S], F16)
        kl16 = big.tile([128, S], F16)

        # ================= Phase A: QKV projections =================
        # feature-major: psum[f 128, m 512] += w[kb,f].T @ xT[kb, mquarter]
        for mq_ in range(4):
            ms = slice(mq_ * 512, (mq_ + 1) * 512)
            pq = [psum.tile([128, 512], F32, tag="t512", bufs=4, name=f"pq{fb}_{mq_}")
                  for fb in range(HPC)]
            pk = psum.tile([128, 512], F32, tag="t512", bufs=4)
            pv = psum.tile([128, 512], F32, tag="t512", bufs=4)
            for kb in range(NKB):
                xh_t = xpool.tile([128, 512], F16, tag="xh")
                nc.sync.dma_start(xh_t[:], xth_d[kb * 128:(kb + 1) * 128, ms])
                xl_t = xpool.tile([128, 512], F16, tag="xl")
                nc.sync.dma_start(xl_t[:], xtl_d[kb * 128:(kb + 1) * 128, ms])
                st = kb == 0
                sp = kb == NKB - 1
                for fb in range(HPC):
                    fsl = slice(fb * 128, (fb + 1) * 128)
                    nc.tensor.matmul(pq[fb][:], wqh_sb[:, kb, fsl], xh_t[:],
                                     start=st, stop=False)
                    nc.tensor.matmul(pq[fb][:], wqh_sb[:, kb, fsl], xl_t[:],
                                     start=False, stop=False)
                    nc.tensor.matmul(pq[fb][:], wql_sb[:, kb, fsl], xh_t[:],
                                     start=False, stop=sp)
                nc.tensor.matmul(pk[:], wkh_sb[:, kb, :], xh_t[:],
                                 start=st, stop=False)
                nc.tensor.matmul(pk[:], wkh_sb[:, kb, :], xl_t[:],
                                 start=False, stop=False)
                nc.tensor.matmul(pk[:], wkl_sb[:, kb, :], xh_t[:],
                                 start=False, stop=sp)
                nc.tensor.matmul(pv[:], wvh_sb[:, kb, :], xh_t[:],
                                 start=st, stop=sp)
            for fb in range(HPC):
                nc.vector.tensor_scalar_add(qt_f32[:, fb, ms], pq[fb][:],
                                            bqt_sb[:, fb:fb + 1])
            nc.vector.tensor_scalar_add(kt_f32[:, ms], pk[:], bkt_sb[:])
            nc.vector.tensor_scalar_add(vt16[:, ms], pv[:], bvt_sb[:])

        # v: feature-major -> token-major via PE transpose
        for mb in range(NMB):
            pvt = psum.tile([128, 128], F16, tag="t128", bufs=3)
            nc.tensor.transpose(pvt[:], vt16[:, mb * 128:(mb + 1) * 128],
                                ident16[:])
            nc.vector.tensor_copy(v_sb[:, mb], pvt[:])

        # ================= Phase B: RMS + RoPE + split =================
        # rms scale s = sqrt(mean(q^2)+eps) per token (partition-dim reduce
        # via PE ones-matmul); broadcast back via K=1 PE matmul.
        specs = [
            (qt_f32[:, 0], qh16[:, 0], ql16[:, 0], bcq, qn_sb),
            (qt_f32[:, 1], qh16[:, 1], ql16[:, 1], bcq, qn_sb),
            (kt_f32[:], kh16[:], kl16[:], bck, kn_sb),
        ]
        for src, dsth, dstl, bcast, nw in specs:
            for ch in range(NCH):
                sl = slice(ch * 512, (ch + 1) * 512)
                sq = btmp.tile([128, 512], F32, tag="sq")
                nc.scalar.activation(sq[:], src[:, sl], ACTF.Square)
                pss = psum.tile([1, 512], F32, tag="t512", bufs=4)
                nc.tensor.matmul(pss[:], ones_col[:], sq[:],
                                 start=True, stop=True)
                ssb = btmp.tile([1, 512], F32, tag="ssb")
                nc.scalar.activation(ssb[:], pss[:], ACTF.Sqrt,
                                     bias=eps_t[:], scale=1.0 / HD)
                pbc = psum.tile([128, 512], F32, tag="t512", bufs=4)
                nc.tensor.matmul(pbc[:], bcast[:], ssb[:],
                                 start=True, stop=True)
                t1 = btmp.tile([128, 512], F32, tag="t1")
                nc.vector.tensor_mul(t1[:], src[:, sl], pbc[:])
                nc.vector.tensor_scalar_mul(t1[:], t1[:], nw[:])
                rot = btmp.tile([128, 512], F32, tag="rot")
                nc.vector.tensor_scalar_mul(rot[0:64, :], t1[64:128, :], -1.0)
                nc.vector.tensor_copy(rot[64:128, :], t1[0:64, :])
                qr = btmp.tile([128, 512], F32, tag="qr")
                nc.vector.tensor_mul(qr[:], t1[:], cost_sb[:, sl])
                nc.vector.tensor_mul(rot[:], rot[:], sint_sb[:, sl])
                nc.vector.tensor_add(qr[:], qr[:], rot[:])
                nc.vector.tensor_copy(dsth[:, sl], qr[:])
                nc.vector.tensor_sub(dstl[:, sl], qr[:], dsth[:, sl])

        # ================= Phase C/D: attention + O proj =================
        for i in range(NMB):
            nchunks = (i // 4 + 1) if is_causal else NCH
            attn16 = cpool.tile([128, HPC, 128], F16, tag="attn16")
            s_sbs, negms, lpartss, out_pss = [], [], [], []
            out_ps2_shared = [None]
            # pass 1 (both heads): scores (3x fp16 split matmuls), row maxes
            for h in range(HPC):
                qh_blk = qh16[:, h, i * 128:(i + 1) * 128]
                ql_blk = ql16[:, h, i * 128:(i + 1) * 128]
                s_sb = cpool.tile([128, NCH, 512], F32, tag="s_sb",
                                  name=f"s_sb_{i}_{h}")
                for ncj in range(nchunks):
                    ks = slice(ncj * 512, (ncj + 1) * 512)
                    ps_s = psum.tile([128, 512], F32, tag="t512", bufs=4,
                                     name=f"ps_s_{i}_{h}_{ncj}")
                    nc.tensor.matmul(ps_s[:], qh_blk, kh16[:, ks],
                                     start=True, stop=False)
                    nc.tensor.matmul(ps_s[:], qh_blk, kl16[:, ks],
                                     start=False, stop=False)
                    nc.tensor.matmul(ps_s[:], ql_blk, kh16[:, ks],
                                     start=False, stop=True)
                    if is_causal and ncj == i // 4:
                        nc.vector.tensor_add(s_sb[:, ncj, :], ps_s[:],
                                             cmask[:, i % 4, :])
                    elif not is_causal:
                        mload = cpool.tile([128, 512], F32, tag="mload",
                                           bufs=3, name=f"mload_{i}_{h}_{ncj}")
                        nc.sync.dma_start(
                            mload[:], mask_d[i * 128:(i + 1) * 128, ks])
                        nc.vector.tensor_add(s_sb[:, ncj, :], ps_s[:],
                                             mload[:])
                    else:
                        nc.scalar.copy(s_sb[:, ncj, :], ps_s[:])
                negm = cpool.tile([128, 1], F32, tag="negm",
                                  name=f"negm_{i}_{h}")
                nc.vector.reduce_max(negm[:], s_sb[:, 0:nchunks, :],
                                     axis=mybir.AxisListType.XY, negate=True)
                s_sbs.append(s_sb)
                negms.append(negm)
            # pass 2 (both heads): exp (fp16) -> PE transpose -> PV accumulate
            for h in range(HPC):
                s_sb, negm = s_sbs[h], negms[h]
                lparts = cpool.tile([128, NCH], F32, tag="lparts",
                                    name=f"lparts_{i}_{h}")
                if h == 0:
                    out_ps2 = psum.tile([128, HPC, 128], F32, tag="t128",
                                        bufs=3, name=f"out_ps2_{i}")
                    out_ps2_shared[0] = out_ps2
                out_ps = out_ps2_shared[0][:, h, :]
                last_nkb = i if is_causal else nchunks * 4 - 1
                for ncj in range(nchunks):
                    if ncj % 2 == 0:
                        w = min(2, nchunks - ncj)
                        p16w = cpool.tile([128, 2, 512], F16, tag="p16",
                                          name=f"p16_{i}_{h}_{ncj}")
                        nc.scalar.activation(
                            p16w[:, 0:w, :], s_sb[:, ncj:ncj + w, :],
                            ACTF.Exp, bias=negm[:], scale=1.0,
                            accum_out=lparts[:, ncj // 2:ncj // 2 + 1])
                    p16 = p16w[:, ncj % 2, :]
                    nb = min(4, last_nkb + 1 - ncj * 4)
                    ps_t4 = psum.tile([128, 4, 128], F16, tag="t128", bufs=3,
                                      name=f"ps_t4_{i}_{h}_{ncj}")
                    for b in range(nb):
                        nc.tensor.transpose(
                            ps_t4[:, b, :], p16[:, b * 128:(b + 1) * 128],
                            ident16[:])
                    pt_sb = cpool.tile([128, 4, 128], F16, tag="pt_sb",
                                       bufs=4, name=f"pt_sb_{i}_{h}_{ncj}")
                    if ncj % 2 == 0:
                        nc.vector.tensor_copy(pt_sb[:, 0:nb, :],
                                              ps_t4[:, 0:nb, :])
                    else:
                        nc.scalar.copy(pt_sb[:, 0:nb, :], ps_t4[:, 0:nb, :])
                    for b in range(nb):
                        nkb = ncj * 4 + b
                        nc.tensor.matmul(out_ps, pt_sb[:, b, :],
                                         v_sb[:, nkb],
                                         start=(nkb == 0),
                                         stop=(nkb == last_nkb))
                lpartss.append(lparts)
                out_pss.append(out_ps)
            for h in range(HPC):
                lsum = cpool.tile([128, 1], F32, tag="lsum",
                                  name=f"lsum_{i}_{h}")
                nc.vector.reduce_sum(lsum[:], lpartss[h][:, 0:(nchunks + 1) // 2],
                                     axis=AX)
                linv = cpool.tile([128, 1], F32, tag="linv",
                                  name=f"linv_{i}_{h}")
                nc.vector.reciprocal(linv[:], lsum[:])
                at = cpool.tile([128, 128], F16, tag="at", name=f"at_{i}_{h}")
                nc.vector.tensor_scalar_mul(at[:], out_pss[h], linv[:])
                pat = psum.tile([128, 128], F16, tag="t128", bufs=3,
                                name=f"pat_{i}_{h}")
                nc.tensor.transpose(pat[:], at[:], ident16[:])
                nc.vector.tensor_copy(attn16[:, h], pat[:])
            # O proj partial: out[m, n] += attnT[f, m].T @ wo[f, n]
            for nh_ in range(4):
                ns = slice(nh_ * 512, (nh_ + 1) * 512)
                po = psum.tile([128, 512], F32, tag="pod", bufs=1,
                               name=f"po_{i}_{nh_}")
                nc.tensor.matmul(po[:], attn16[:, 0], woh_sb[:, 0, ns],
                                 start=True, stop=False)
                nc.tensor.matmul(po[:], attn16[:, 1], woh_sb[:, 1, ns],
                                 start=False, stop=True)
                ob = dpool.tile([128, 512], F32, tag="ob",
                                name=f"ob_{i}_{nh_}")
                if nh_ % 2 == 0:
                    nc.vector.tensor_copy(ob[:], po[:])
                else:
                    nc.scalar.copy(ob[:], po[:])
                nc.sync.dma_start(out_d[i * 128:(i + 1) * 128, ns], ob[:])

    nc.compile()
    return nc


def _split16(a):
    hi = a.astype(np.float16)
    lo = (a - hi.astype(np.float32)).astype(np.float16)
    return hi, lo


def kernel(**inputs):
    x = np.asarray(inputs["x"], np.float32)
    cos = np.asarray(inputs["cos"], np.float32)
    sin = np.asarray(inputs["sin"], np.float32)
    am = np.asarray(inputs["attention_mask"]).reshape(S, S).astype(bool)
    wq = np.asarray(inputs["wq"], np.float32)
    bq = np.asarray(inputs["bq"], np.float32)
    wk = np.asarray(inputs["wk"], np.float32)
    bk = np.asarray(inputs["bk"], np.float32)
    wv = np.asarray(inputs["wv"], np.float32)
    bv = np.asarray(inputs["bv"], np.float32)
    wo = np.asarray(inputs["wo"], np.float32)
    bo = np.asarray(inputs["bo"], np.float32)
    qn = np.asarray(inputs["q_norm_w"], np.float32)
    kn = np.asarray(inputs["k_norm_w"], np.float32)

    assert x.shape == (1, S, H)
    is_causal = bool(
        (am == np.triu(np.ones((S, S), dtype=bool), k=1)).all())

    key = is_causal
    if key not in _prog_cache:
        _prog_cache[key] = _build(is_causal)
    nc = _prog_cache[key]

    xT = np.ascontiguousarray(x[0].T)
    xth, xtl = _split16(xT)
    cosT = np.ascontiguousarray(cos.T)
    sinT = np.ascontiguousarray(sin.T)
    qn_c = np.ascontiguousarray(qn.reshape(HD, 1))
    kn_c = np.ascontiguousarray(kn.reshape(HD, 1))
    if not is_causal:
        maskadd = np.where(am, np.float32(NEG), np.float32(0.0))

    in_maps = []
    for c in range(NCORES):
        fq = slice(c * FQ, (c + 1) * FQ)
        g = c // 2
        fk = slice(g * HD, (g + 1) * HD)
        wqh, wql = _split16(wq[:, fq])
        wkh, wkl = _split16(wk[:, fk])
        m = dict(
            xth=xth, xtl=xtl,
            wqh=np.ascontiguousarray(wqh), wql=np.ascontiguousarray(wql),
            wkh=np.ascontiguousarray(wkh), wkl=np.ascontiguousarray(wkl),
            wvh=np.ascontiguousarray(wv[:, fk].astype(np.float16)),
            woh=np.ascontiguousarray(wo[fq, :].astype(np.float16)),
            cost=cosT, sint=sinT,
            bqt=np.ascontiguousarray(bq[fq].reshape(HPC, HD).T),
            bkt=np.ascontiguousarray(bk[fk].reshape(1, HD).T),
            bvt=np.ascontiguousarray(bv[fk].reshape(1, HD).T),
            qn=qn_c, kn=kn_c,
        )
        if not is_causal:
            m["maskadd"] = maskadd
        in_maps.append(m)

    res = bass_utils.run_bass_kernel_spmd(nc, in_maps,
                                          core_ids=list(range(NCORES)))
    acc = np.zeros((S, H), np.float64)
    for c in range(NCORES):
        acc += res.results[c]["out"]
    out = (acc + bo[None, :]).astype(np.float32)
    return out.reshape(1, S, H)


# revision 19
# speedup vs baseline: 25302.4547x; 23253.6099x over previous
"""Trainium2 Bass kernel for nn_Attention_89902255440825.

Single-layer attention block: QKV proj + per-head RMS("mult" variant) +
RoPE + GQA causal attention with softmax(scores * sqrt(HD)) + O proj.

Sharding (8 NeuronCores, tensor-parallel over heads):
  core c: q heads {2c, 2c+1}  (wq cols 256c:256c+256)
          kv head c//2        (wk/wv cols 128*(c//2):...)
          wo rows 256c:256c+256  -> partial [S,H] outputs, summed on host.

Precision strategy (measured on HW):
  - q/k path (projections + scores) uses fp16 hi/lo split: 3 fp16 matmuls
    give ~fp32 accuracy at 3 cyc/row (native fp32 is 4 cyc/row).
    Required because softmax here is multiplied by sqrt(HD): logit std ~95,
    near-argmax attention, so logits need ~1e-5 relative accuracy.
  - v / PV / O-proj use single-pass fp16 (error ~1e-3, benign).

Layouts on device (per core):
  xT (hi/lo fp16)      [H=2048, S=2048]   host-pretransposed
  qT/kT feature-major  [d=128, m=2048]    d on partitions (scores lhsT/rhs)
  v    token-major     [m, d] tiles       (PV rhs)
  attn_outT via PE transpose -> O proj lhsT
"""
import numpy as np
from contextlib import ExitStack

import concourse.bass as bass
import concourse.tile as tile
from concourse import bacc, mybir, bass_utils
from concourse.masks import make_identity

S = 2048
H = 2048
HD = 128
NH = 16
NKV = 4
NCORES = 8
HPC = NH // NCORES          # q heads per core = 2
FQ = HPC * HD               # q features per core = 256
EPS = 1e-6
SM_SCALE = float(np.sqrt(HD))   # reference: softmax(scores / HD**-0.5)
NEG = -1e30
F32 = mybir.dt.float32
F16 = mybir.dt.float16
AX = mybir.AxisListType.X
OP = mybir.AluOpType
ACTF = mybir.ActivationFunctionType

_prog_cache = {}


def _build(is_causal: bool):
    nc = bacc.Bacc("TRN2", target_bir_lowering=False, debug=False,
                   num_devices=NCORES)

    def din(name, shape, dt=F16):
        return nc.dram_tensor(name, shape, dt, kind="ExternalInput").ap()

    xth_d = din("xth", [H, S])
    xtl_d = din("xtl", [H, S])
    wqh_d = din("wqh", [H, FQ])
    wql_d = din("wql", [H, FQ])
    wkh_d = din("wkh", [H, HD])
    wkl_d = din("wkl", [H, HD])
    wvh_d = din("wvh", [H, HD])
    woh_d = din("woh", [FQ, H])
    cosq_d = din("cosq", [HD, S], F32)
    sinq_d = din("sinq", [HD, S], F32)
    cosk_d = din("cosk", [HD, S], F32)
    sink_d = din("sink", [HD, S], F32)
    bqt_d = din("bqt", [HD, HPC], F32)
    bkt_d = din("bkt", [HD, 1], F32)
    bvt_d = din("bvt", [HD, 1], F32)
    if not is_causal:
        mask_d = din("maskadd", [S, S], F32)
    out_d = nc.dram_tensor("out", [S, H], F32, kind="ExternalOutput").ap()

    NKB = H // 128            # 16 contraction k-blocks
    NMB = S // 128            # 16 token blocks
    NCH = S // 512            # 4 512-chunks

    with tile.TileContext(nc) as tc, ExitStack() as ctx:
        const = ctx.enter_context(tc.tile_pool(name="const", bufs=1))
        wpool = ctx.enter_context(tc.tile_pool(name="wpool", bufs=1))
        big = ctx.enter_context(tc.tile_pool(name="big", bufs=1))
        xpool = ctx.enter_context(tc.tile_pool(name="xpool", bufs=3))
        btmp = ctx.enter_context(tc.tile_pool(name="btmp", bufs=2))
        cpool = ctx.enter_context(tc.tile_pool(name="cpool", bufs=2))
        dpool = ctx.enter_context(tc.tile_pool(name="dpool", bufs=3))
        psum = ctx.enter_context(tc.tile_pool(name="psum", bufs=1, space="PSUM"))
        dscr = ctx.enter_context(tc.tile_pool(name="dscr", bufs=3, space="DRAM"))

        # ---- constants ----
        ident16 = const.tile([128, 128], F16)
        make_identity(nc, ident16[:])
        ones_col = const.tile([128, 1], F32)
        nc.vector.memset(ones_col[:], 1.0)
        eps_q = const.tile([1, 1], F32)   # q: 11.31*sqrt(x/128+eps) = sqrt(x+128*eps)
        nc.vector.memset(eps_q[:], EPS * HD)
        eps_k = const.tile([1, 1], F32)
        nc.vector.memset(eps_k[:], EPS)
        if is_causal:
            cmask = const.tile([128, 4, 512], F32)
            for r in range(4):
                nc.vector.memset(cmask[:, r, :], 0.0)
                nc.gpsimd.affine_select(
                    out=cmask[:, r, :], in_=cmask[:, r, :],
                    compare_op=OP.is_ge, fill=NEG,
                    base=128 * r, channel_multiplier=1, pattern=[[-1, 512]],
                )

        # ---- weights / small inputs to SBUF ----
        def wtiles(dram, f, nm):
            t = wpool.tile([128, NKB, f], F16, name=nm, tag=nm)
            nc.sync.dma_start(t[:], dram.rearrange("(t p) f -> p t f", p=128))
            return t

        wqh_sb = wtiles(wqh_d, FQ, "wqh_sb")
        wql_sb = wtiles(wql_d, FQ, "wql_sb")
        wkh_sb = wtiles(wkh_d, HD, "wkh_sb")
        wkl_sb = wtiles(wkl_d, HD, "wkl_sb")
        wvh_sb = wtiles(wvh_d, HD, "wvh_sb")
        woh_sb = wpool.tile([128, HPC, H], F16)
        nc.sync.dma_start(woh_sb[:], woh_d.rearrange("(t p) f -> p t f", p=128))
        cosq_sb = wpool.tile([HD, S], F32)
        nc.sync.dma_start(cosq_sb[:], cosq_d)
        sinq_sb = wpool.tile([HD, S], F32)
        nc.sync.dma_start(sinq_sb[:], sinq_d)
        cosk_sb = wpool.tile([HD, S], F32)
        nc.sync.dma_start(cosk_sb[:], cosk_d)
        sink_sb = wpool.tile([HD, S], F32)
        nc.sync.dma_start(sink_sb[:], sink_d)
        bqt_sb = wpool.tile([HD, HPC], F32)
        nc.sync.dma_start(bqt_sb[:], bqt_d)
        bkt_sb = wpool.tile([HD, 1], F32)
        nc.sync.dma_start(bkt_sb[:], bkt_d)
        bvt_sb = wpool.tile([HD, 1], F32)
        nc.sync.dma_start(bvt_sb[:], bvt_d)

        # ---- persistent activations ----
        qt_f32 = big.tile([128, HPC, S], F32)     # q feature-major, pre-norm
        kt_f32 = big.tile([128, S], F32)
        vt16 = big.tile([128, S], F16)            # v feature-major fp16
        v_sb = big.tile([128, NMB, 128], F16)     # v token-major fp16
        qh16 = big.tile([128, HPC, S], F16)       # roped+scaled q hi/lo
        ql16 = big.tile([128, HPC, S], F16)
        kh16 = big.tile([128, S], F16)
        kl16 = big.tile([128, S], F16)

        # ================= Phase A: QKV projections =================
        # feature-major: psum[f 128, m 512] += w[kb,f].T @ xT[kb, mquarter]
        for mq_ in range(4):
            ms = slice(mq_ * 512, (mq_ + 1) * 512)
            pq = [psum.tile([128, 512], F32, tag="t512", bufs=4, name=f"pq{fb}_{mq_}")
                  for fb in range(HPC)]
            pk = psum.tile([128, 512], F32, tag="t512", bufs=4)
            pv = psum.tile([128, 512], F32, tag="t512", bufs=4)
            for kb in range(NKB):
                xh_t = xpool.tile([128, 512], F16, tag="xh")
                nc.sync.dma_start(xh_t[:], xth_d[kb * 128:(kb + 1) * 128, ms])
                xl_t = xpool.tile([128, 512], F16, tag="xl")
                nc.sync.dma_start(xl_t[:], xtl_d[kb * 128:(kb + 1) * 128, ms])
                st = kb == 0
                sp = kb == NKB - 1
                for fb in range(HPC):
                    fsl = slice(fb * 128, (fb + 1) * 128)
                    nc.tensor.matmul(pq[fb][:], wqh_sb[:, kb, fsl], xh_t[:],
                                     start=st, stop=False)
                    nc.tensor.matmul(pq[fb][:], wqh_sb[:, kb, fsl], xl_t[:],
                                     start=False, stop=False)
                    nc.tensor.matmul(pq[fb][:], wql_sb[:, kb, fsl], xh_t[:],
                                     start=False, stop=sp)
                nc.tensor.matmul(pk[:], wkh_sb[:, kb, :], xh_t[:],
                                 start=st, stop=False)
                nc.tensor.matmul(pk[:], wkh_sb[:, kb, :], xl_t[:],
                                 start=False, stop=False)
                nc.tensor.matmul(pk[:], wkl_sb[:, kb, :], xh_t[:],
                                 start=False, stop=sp)
                nc.tensor.matmul(pv[:], wvh_sb[:, kb, :], xh_t[:],
                                 start=st, stop=sp)
            for fb in range(HPC):
                nc.vector.tensor_scalar_add(qt_f32[:, fb, ms], pq[fb][:],
                                            bqt_sb[:, fb:fb + 1])
            nc.vector.tensor_scalar_add(kt_f32[:, ms], pk[:], bkt_sb[:])
            nc.vector.tensor_scalar_add(vt16[:, ms], pv[:], bvt_sb[:])

        # v: feature-major -> token-major via PE transpose
        for mb in range(NMB):
            pvt = psum.tile([128, 128], F16, tag="t128", bufs=3)
            nc.tensor.transpose(pvt[:], vt16[:, mb * 128:(mb + 1) * 128],
                                ident16[:])
            nc.vector.tensor_copy(v_sb[:, mb], pvt[:])

        # ================= Phase B: RMS + RoPE + split =================
        # rms scale s = sqrt(mean(q^2)+eps) per token (partition-dim reduce
        # via PE ones-matmul); broadcast back via K=1 PE matmul.
        specs = [
            (kt_f32[:], kh16[:], kl16[:], eps_k, 1.0 / HD, cosk_sb, sink_sb),
            (qt_f32[:, 0], qh16[:, 0], ql16[:, 0], eps_q, 1.0, cosq_sb, sinq_sb),
            (qt_f32[:, 1], qh16[:, 1], ql16[:, 1], eps_q, 1.0, cosq_sb, sinq_sb),
        ]
        for src, dsth, dstl, epst, sscale, cos_sb, sin_sb in specs:
            for ch in range(NCH):
                sl = slice(ch * 512, (ch + 1) * 512)
                sq = btmp.tile([128, 512], F32, tag="sq")
                nc.scalar.activation(sq[:], src[:, sl], ACTF.Square)
                pss = psum.tile([1, 512], F32, tag="t512", bufs=4)
                nc.tensor.matmul(pss[:], ones_col[:], sq[:],
                                 start=True, stop=True)
                ssb = btmp.tile([1, 512], F32, tag="ssb")
                nc.scalar.activation(ssb[:], pss[:], ACTF.Sqrt,
                                     bias=epst[:], scale=sscale)
                sdr = dscr.tile([1, 512], F32, tag="sdr")
                nc.sync.dma_start(sdr[:], ssb[:])
                sbc = btmp.tile([128, 512], F32, tag="sbc")
                nc.sync.dma_start(
                    sbc[:], bass.AP(tensor=sdr[:].tensor, offset=sdr[:].offset,
                                    ap=[[0, 128]] + sdr[:].ap[1:]))
                t1 = btmp.tile([128, 512], F32, tag="t1")
                nc.vector.tensor_mul(t1[:], src[:, sl], sbc[:])
                rot = btmp.tile([128, 512], F32, tag="rot")
                nc.vector.tensor_scalar_mul(rot[0:64, :], t1[64:128, :], -1.0)
                nc.vector.tensor_copy(rot[64:128, :], t1[0:64, :])
                qr = btmp.tile([128, 512], F32, tag="qr")
                nc.vector.tensor_mul(qr[:], t1[:], cos_sb[:, sl])
                nc.vector.tensor_mul(rot[:], rot[:], sin_sb[:, sl])
                nc.vector.tensor_add(qr[:], qr[:], rot[:])
                nc.scalar.copy(dsth[:, sl], qr[:])
                nc.vector.tensor_sub(dstl[:, sl], qr[:], dsth[:, sl])

        # ================= Phase C/D: attention + O proj =================
        for i in range(NMB):
            nchunks = (i // 4 + 1) if is_causal else NCH
            attn16 = cpool.tile([128, HPC, 128], F16, tag="attn16")
            s_sbs, negms, lpartss, out_pss = [], [], [], []
            out_ps2_shared = [None]
            # pass 1 (both heads): scores (3x fp16 split matmuls), row maxes
            for h in range(HPC):
                qh_blk = qh16[:, h, i * 128:(i + 1) * 128]
                ql_blk = ql16[:, h, i * 128:(i + 1) * 128]
                s_sb = cpool.tile([128, NCH, 512], F32, tag="s_sb",
                                  name=f"s_sb_{i}_{h}")
                for ncj in range(nchunks):
                    ks = slice(ncj * 512, (ncj + 1) * 512)
                    ps_s = psum.tile([128, 512], F32, tag="t512", bufs=4,
                                     name=f"ps_s_{i}_{h}_{ncj}")
                    nc.tensor.matmul(ps_s[:], qh_blk, kh16[:, ks],
                                     start=True, stop=False)
                    nc.tensor.matmul(ps_s[:], qh_blk, kl16[:, ks],
                                     start=False, stop=False)
                    nc.tensor.matmul(ps_s[:], ql_blk, kh16[:, ks],
                                     start=False, stop=True)
                    if is_causal and ncj == i // 4:
                        nc.vector.tensor_add(s_sb[:, ncj, :], ps_s[:],
                                             cmask[:, i % 4, :])
                    elif not is_causal:
                        mload = cpool.tile([128, 512], F32, tag="mload",
                                           bufs=3, name=f"mload_{i}_{h}_{ncj}")
                        nc.sync.dma_start(
                            mload[:], mask_d[i * 128:(i + 1) * 128, ks])
                        nc.vector.tensor_add(s_sb[:, ncj, :], ps_s[:],
                                             mload[:])
                    else:
                        nc.scalar.copy(s_sb[:, ncj, :], ps_s[:])
                negm = cpool.tile([128, 1], F32, tag="negm",
                                  name=f"negm_{i}_{h}")
                nc.vector.reduce_max(negm[:], s_sb[:, 0:nchunks, :],
                                     axis=mybir.AxisListType.XY, negate=True)
                s_sbs.append(s_sb)
                negms.append(negm)
            # pass 2 (both heads): exp (fp16) -> PE transpose -> PV accumulate
            for h in range(HPC):
                s_sb, negm = s_sbs[h], negms[h]
                lparts = cpool.tile([128, NCH], F32, tag="lparts",
                                    name=f"lparts_{i}_{h}")
                if h == 0:
                    out_ps2 = psum.tile([128, HPC, 128], F32, tag="t128",
                                        bufs=3, name=f"out_ps2_{i}")
                    out_ps2_shared[0] = out_ps2
                out_ps = out_ps2_shared[0][:, h, :]
                last_nkb = i if is_causal else nchunks * 4 - 1
                for ncj in range(nchunks):
                    if ncj % 2 == 0:
                        w = min(2, nchunks - ncj)
                        p16w = cpool.tile([128, 2, 512], F16, tag="p16",
                                          name=f"p16_{i}_{h}_{ncj}")
                        nc.scalar.activation(
                            p16w[:, 0:w, :], s_sb[:, ncj:ncj + w, :],
                            ACTF.Exp, bias=negm[:], scale=1.0,
                            accum_out=lparts[:, ncj // 2:ncj // 2 + 1])
                    p16 = p16w[:, ncj % 2, :]
                    nb = min(4, last_nkb + 1 - ncj * 4)
                    ps_t4 = psum.tile([128, 4, 128], F16, tag="t128", bufs=3,
                                      name=f"ps_t4_{i}_{h}_{ncj}")
                    for b in range(nb):
                        nc.tensor.transpose(
                            ps_t4[:, b, :], p16[:, b * 128:(b + 1) * 128],
                            ident16[:])
                    pt_sb = cpool.tile([128, 4, 128], F16, tag="pt_sb",
                                       bufs=4, name=f"pt_sb_{i}_{h}_{ncj}")
                    if ncj % 2 == 0:
                        nc.vector.tensor_copy(pt_sb[:, 0:nb, :],
                                              ps_t4[:, 0:nb, :])
                    else:
                        nc.scalar.copy(pt_sb[:, 0:nb, :], ps_t4[:, 0:nb, :])
                    for b in range(nb):
                        nkb = ncj * 4 + b
                        nc.tensor.matmul(out_ps, pt_sb[:, b, :],
                                         v_sb[:, nkb],
                                         start=(nkb == 0),
                                         stop=(nkb == last_nkb))
                lpartss.append(lparts)
                out_pss.append(out_ps)
            for h in range(HPC):
                lsum = cpool.tile([128, 1], F32, tag="lsum",
                                  name=f"lsum_{i}_{h}")
                nc.vector.reduce_sum(lsum[:], lpartss[h][:, 0:(nchunks + 1) // 2],
                                     axis=AX)
                linv = cpool.tile([128, 1], F32, tag="linv",
                                  name=f"linv_{i}_{h}")
                nc.vector.reciprocal(linv[:], lsum[:])
                at = cpool.tile([128, 128], F16, tag="at", name=f"at_{i}_{h}")
                nc.vector.tensor_scalar_mul(at[:], out_pss[h], linv[:])
                pat = psum.tile([128, 128], F16, tag="t128", bufs=3,
                                name=f"pat_{i}_{h}")
                nc.tensor.transpose(pat[:], at[:], ident16[:])
                nc.vector.tensor_copy(attn16[:, h], pat[:])
            # O proj partial: out[m, n] += attnT[f, m].T @ wo[f, n]
            for nh_ in range(4):
                ns = slice(nh_ * 512, (nh_ + 1) * 512)
                po = psum.tile([128, 512], F32, tag="pod", bufs=1,
                               name=f"po_{i}_{nh_}")
                nc.tensor.matmul(po[:], attn16[:, 0], woh_sb[:, 0, ns],
                                 start=True, stop=False)
                nc.tensor.matmul(po[:], attn16[:, 1], woh_sb[:, 1, ns],
                                 start=False, stop=True)
                ob = dpool.tile([128, 512], F32, tag="ob",
                                name=f"ob_{i}_{nh_}")
                if nh_ % 2 == 0:
                    nc.vector.tensor_copy(ob[:], po[:])
                else:
                    nc.scalar.copy(ob[:], po[:])
                nc.sync.dma_start(out_d[i * 128:(i + 1) * 128, ns], ob[:])

    nc.compile()
    return nc


def _split16(a):
    hi = a.astype(np.float16)
    lo = (a - hi.astype(np.float32)).astype(np.float16)
    return hi, lo


def kernel(**inputs):
    x = np.asarray(inputs["x"], np.float32)
    cos = np.asarray(inputs["cos"], np.float32)
    sin = np.asarray(inputs["sin"], np.float32)
    am = np.asarray(inputs["attention_mask"]).reshape(S, S).astype(bool)
    wq = np.asarray(inputs["wq"], np.float32)
    bq = np.asarray(inputs["bq"], np.float32)
    wk = np.asarray(inputs["wk"], np.float32)
    bk = np.asarray(inputs["bk"], np.float32)
    wv = np.asarray(inputs["wv"], np.float32)
    bv = np.asarray(inputs["bv"], np.float32)
    wo = np.asarray(inputs["wo"], np.float32)
    bo = np.asarray(inputs["bo"], np.float32)
    qn = np.asarray(inputs["q_norm_w"], np.float32)
    kn = np.asarray(inputs["k_norm_w"], np.float32)

    assert x.shape == (1, S, H)
    is_causal = bool(
        (am == np.triu(np.ones((S, S), dtype=bool), k=1)).all())

    key = is_causal
    if key not in _prog_cache:
        _prog_cache[key] = _build(is_causal)
    nc = _prog_cache[key]

    xT = np.ascontiguousarray(x[0].T)
    xth, xtl = _split16(xT)
    cosT = cos.T
    sinT = sin.T
    rolled_q = np.roll(qn, -64)     # rot(q*qn)[i] = rot(q)[i] * qn[(i+64)%128]
    rolled_k = np.roll(kn, -64)
    cosq = np.ascontiguousarray(cosT * qn[:, None])
    sinq = np.ascontiguousarray(sinT * rolled_q[:, None])
    cosk = np.ascontiguousarray(cosT * kn[:, None])
    sink = np.ascontiguousarray(sinT * rolled_k[:, None])
    if not is_causal:
        maskadd = np.where(am, np.float32(NEG), np.float32(0.0))

    in_maps = []
    for c in range(NCORES):
        fq = slice(c * FQ, (c + 1) * FQ)
        g = c // 2
        fk = slice(g * HD, (g + 1) * HD)
        wqh, wql = _split16(wq[:, fq])
        wkh, wkl = _split16(wk[:, fk])
        m = dict(
            xth=xth, xtl=xtl,
            wqh=np.ascontiguousarray(wqh), wql=np.ascontiguousarray(wql),
            wkh=np.ascontiguousarray(wkh), wkl=np.ascontiguousarray(wkl),
            wvh=np.ascontiguousarray(wv[:, fk].astype(np.float16)),
            woh=np.ascontiguousarray(wo[fq, :].astype(np.float16)),
            cosq=cosq, sinq=sinq, cosk=cosk, sink=sink,
            bqt=np.ascontiguousarray(bq[fq].reshape(HPC, HD).T),
            bkt=np.ascontiguousarray(bk[fk].reshape(1, HD).T),
            bvt=np.ascontiguousarray(bv[fk].reshape(1, HD).T),
        )
        if not is_causal:
            m["maskadd"] = maskadd
        in_maps.append(m)

    res = bass_utils.run_bass_kernel_spmd(nc, in_maps,
                                          core_ids=list(range(NCORES)))
    acc = np.zeros((S, H), np.float64)
    for c in range(NCORES):
        acc += res.results[c]["out"]
    out = (acc + bo[None, :]).astype(np.float32)
    return out.reshape(1, S, H)


# revision 23
# speedup vs baseline: 26007.2098x; 1.0279x over previous
"""Trainium2 Bass kernel for nn_Attention_89902255440825.

Single-layer attention block: QKV proj + per-head RMS("mult" variant) +
RoPE + GQA causal attention with softmax(scores * sqrt(HD)) + O proj.

Sharding (8 NeuronCores, tensor-parallel over heads):
  core c: q heads {2c, 2c+1}  (wq cols 256c:256c+256)
          kv head c//2        (wk/wv cols 128*(c//2):...)
          wo rows 256c:256c+256  -> partial [S,H] outputs, summed on host.

Precision strategy (measured on HW):
  - q/k path (projections + scores) uses fp16 hi/lo split: 3 fp16 matmuls
    give ~fp32 accuracy at 3 cyc/row (native fp32 is 4 cyc/row).
    Required because softmax here is multiplied by sqrt(HD): logit std ~95,
    near-argmax attention, so logits need ~1e-5 relative accuracy.
  - v / PV / O-proj use single-pass fp16 (error ~1e-3, benign).

Layouts on device (per core):
  xT (hi/lo fp16)      [H=2048, S=2048]   host-pretransposed
  qT/kT feature-major  [d=128, m=2048]    d on partitions (scores lhsT/rhs)
  v    token-major     [m, d] tiles       (PV rhs)
  attn_outT via PE transpose -> O proj lhsT
"""
import numpy as np
from contextlib import ExitStack

import concourse.bass as bass
import concourse.tile as tile
from concourse import bacc, mybir, bass_utils
from concourse.masks import make_identity

S = 2048
H = 2048
HD = 128
NH = 16
NKV = 4
NCORES = 8
HPC = NH // NCORES          # q heads per core = 2
FQ = HPC * HD               # q features per core = 256
EPS = 1e-6
SM_SCALE = float(np.sqrt(HD))   # reference: softmax(scores / HD**-0.5)
NEG = -1e30
F32 = mybir.dt.float32
F16 = mybir.dt.float16
AX = mybir.AxisListType.X
OP = mybir.AluOpType
ACTF = mybir.ActivationFunctionType

_prog_cache = {}


def _build(is_causal: bool):
    nc = bacc.Bacc("TRN2", target_bir_lowering=False, debug=False,
                   num_devices=NCORES)

    def din(name, shape, dt=F16):
        return nc.dram_tensor(name, shape, dt, kind="ExternalInput").ap()

    xth_d = din("xth", [H, S])
    xtl_d = din("xtl", [H, S])
    wqh_d = din("wqh", [H, FQ])
    wql_d = din("wql", [H, FQ])
    wkh_d = din("wkh", [H, HD])
    wkl_d = din("wkl", [H, HD])
    wvh_d = din("wvh", [H, HD])
    woh_d = din("woh", [FQ, H])
    cosq_d = din("cosq", [HD, S], F32)
    sinq_d = din("sinq", [HD, S], F32)
    cosk_d = din("cosk", [HD, S], F32)
    sink_d = din("sink", [HD, S], F32)
    bqt_d = din("bqt", [HD, HPC], F32)
    bkt_d = din("bkt", [HD, 1], F32)
    bvt_d = din("bvt", [HD, 1], F32)
    if not is_causal:
        mask_d = din("maskadd", [S, S], F32)
    out_d = nc.dram_tensor("out", [S, H], F32, kind="ExternalOutput").ap()

    NKB = H // 128            # 16 contraction k-blocks
    NMB = S // 128            # 16 token blocks
    NCH = S // 512            # 4 512-chunks

    with tile.TileContext(nc) as tc, ExitStack() as ctx:
        const = ctx.enter_context(tc.tile_pool(name="const", bufs=1))
        wpool = ctx.enter_context(tc.tile_pool(name="wpool", bufs=1))
        big = ctx.enter_context(tc.tile_pool(name="big", bufs=1))
        xpool = ctx.enter_context(tc.tile_pool(name="xpool", bufs=3))
        btmp = ctx.enter_context(tc.tile_pool(name="btmp", bufs=2))
        cpool = ctx.enter_context(tc.tile_pool(name="cpool", bufs=2))
        dpool = ctx.enter_context(tc.tile_pool(name="dpool", bufs=3))
        psum = ctx.enter_context(tc.tile_pool(name="psum", bufs=1, space="PSUM"))
        dscr = ctx.enter_context(tc.tile_pool(name="dscr", bufs=3, space="DRAM"))

        # ---- constants ----
        ident16 = const.tile([128, 128], F16)
        make_identity(nc, ident16[:])
        ones_col = const.tile([128, 1], F32)
        nc.vector.memset(ones_col[:], 1.0)
        eps_q = const.tile([1, 1], F32)   # q: 11.31*sqrt(x/128+eps) = sqrt(x+128*eps)
        nc.vector.memset(eps_q[:], EPS * HD)
        eps_k = const.tile([1, 1], F32)
        nc.vector.memset(eps_k[:], EPS)
        if is_causal:
            cmask = const.tile([128, 4, 512], F32)
            for r in range(4):
                nc.vector.memset(cmask[:, r, :], 0.0)
                nc.gpsimd.affine_select(
                    out=cmask[:, r, :], in_=cmask[:, r, :],
                    compare_op=OP.is_ge, fill=NEG,
                    base=128 * r, channel_multiplier=1, pattern=[[-1, 512]],
                )

        # ---- weights / small inputs to SBUF ----
        def wtiles(dram, f, nm):
            t = wpool.tile([128, NKB, f], F16, name=nm, tag=nm)
            nc.sync.dma_start(t[:], dram.rearrange("(t p) f -> p t f", p=128))
            return t

        wqh_sb = wtiles(wqh_d, FQ, "wqh_sb")
        wql_sb = wtiles(wql_d, FQ, "wql_sb")
        wkh_sb = wtiles(wkh_d, HD, "wkh_sb")
        wkl_sb = wtiles(wkl_d, HD, "wkl_sb")
        wvh_sb = wtiles(wvh_d, HD, "wvh_sb")
        woh_sb = wpool.tile([128, HPC, H], F16)
        cosq_sb = wpool.tile([HD, S], F32)
        sinq_sb = wpool.tile([HD, S], F32)
        cosk_sb = wpool.tile([HD, S], F32)
        sink_sb = wpool.tile([HD, S], F32)
        bqt_sb = wpool.tile([HD, HPC], F32)
        nc.sync.dma_start(bqt_sb[:], bqt_d)
        bkt_sb = wpool.tile([HD, 1], F32)
        nc.sync.dma_start(bkt_sb[:], bkt_d)
        bvt_sb = wpool.tile([HD, 1], F32)
        nc.sync.dma_start(bvt_sb[:], bvt_d)

        # ---- persistent activations ----
        qt_f32 = big.tile([128, HPC, S], F32)     # q feature-major, pre-norm
        kt_f32 = big.tile([128, S], F32)
        vt16 = big.tile([128, S], F16)            # v feature-major fp16
        v_sb = big.tile([128, NMB, 128], F16)     # v token-major fp16
        qh16 = big.tile([128, HPC, S], F16)       # roped+scaled q hi/lo
        ql16 = big.tile([128, HPC, S], F16)
        kh16 = big.tile([128, S], F16)
        kl16 = big.tile([128, S], F16)

        # ================= Phase A: QKV projections =================
        # feature-major: psum[f 128, m 512] += w[kb,f].T @ xT[kb, mquarter]
        for mq_ in range(4):
            ms = slice(mq_ * 512, (mq_ + 1) * 512)
            pq = [psum.tile([128, 512], F32, tag="t512", bufs=4, name=f"pq{fb}_{mq_}")
                  for fb in range(HPC)]
            pk = psum.tile([128, 512], F32, tag="t512", bufs=4)
            pv = psum.tile([128, 512], F32, tag="t512", bufs=4)
            for kb in range(NKB):
                xh_t = xpool.tile([128, 512], F16, tag="xh")
                nc.sync.dma_start(xh_t[:], xth_d[kb * 128:(kb + 1) * 128, ms])
                xl_t = xpool.tile([128, 512], F16, tag="xl")
                nc.sync.dma_start(xl_t[:], xtl_d[kb * 128:(kb + 1) * 128, ms])
                st = kb == 0
                sp = kb == NKB - 1
                for fb in range(HPC):
                    fsl = slice(fb * 128, (fb + 1) * 128)
                    nc.tensor.matmul(pq[fb][:], wqh_sb[:, kb, fsl], xh_t[:],
                                     start=st, stop=False)
                    nc.tensor.matmul(pq[fb][:], wqh_sb[:, kb, fsl], xl_t[:],
                                     start=False, stop=False)
                    nc.tensor.matmul(pq[fb][:], wql_sb[:, kb, fsl], xh_t[:],
                                     start=False, stop=sp)
                nc.tensor.matmul(pk[:], wkh_sb[:, kb, :], xh_t[:],
                                 start=st, stop=False)
                nc.tensor.matmul(pk[:], wkh_sb[:, kb, :], xl_t[:],
                                 start=False, stop=False)
                nc.tensor.matmul(pk[:], wkl_sb[:, kb, :], xh_t[:],
                                 start=False, stop=sp)
                nc.tensor.matmul(pv[:], wvh_sb[:, kb, :], xh_t[:],
                                 start=st, stop=sp)
            for fb in range(HPC):
                nc.vector.tensor_scalar_add(qt_f32[:, fb, ms], pq[fb][:],
                                            bqt_sb[:, fb:fb + 1])
            nc.vector.tensor_scalar_add(kt_f32[:, ms], pk[:], bkt_sb[:])
            nc.vector.tensor_scalar_add(vt16[:, ms], pv[:], bvt_sb[:])

        # deferred loads (not needed until phases B/D; keep A's DMA lanes clear)
        nc.sync.dma_start(cosk_sb[:], cosk_d)
        nc.sync.dma_start(sink_sb[:], sink_d)
        nc.sync.dma_start(cosq_sb[:], cosq_d)
        nc.sync.dma_start(sinq_sb[:], sinq_d)
        nc.sync.dma_start(woh_sb[:], woh_d.rearrange("(t p) f -> p t f", p=128))

        # v: feature-major -> token-major via PE transpose
        for mb in range(NMB):
            pvt = psum.tile([128, 128], F16, tag="t128", bufs=3)
            nc.tensor.transpose(pvt[:], vt16[:, mb * 128:(mb + 1) * 128],
                                ident16[:])
            nc.vector.tensor_copy(v_sb[:, mb], pvt[:])

        # ================= Phase B: RMS + RoPE + split =================
        # rms scale s = sqrt(mean(q^2)+eps) per token (partition-dim reduce
        # via PE ones-matmul); broadcast back via K=1 PE matmul.
        specs = [
            (kt_f32[:], kh16[:], kl16[:], eps_k, 1.0 / HD, cosk_sb, sink_sb),
            (qt_f32[:, 0], qh16[:, 0], ql16[:, 0], eps_q, 1.0, cosq_sb, sinq_sb),
            (qt_f32[:, 1], qh16[:, 1], ql16[:, 1], eps_q, 1.0, cosq_sb, sinq_sb),
        ]
        for src, dsth, dstl, epst, sscale, cos_sb, sin_sb in specs:
            for ch in reversed(range(NCH)):
                sl = slice(ch * 512, (ch + 1) * 512)
                sq = btmp.tile([128, 512], F32, tag="sq")
                nc.scalar.activation(sq[:], src[:, sl], ACTF.Square)
                pss = psum.tile([1, 512], F32, tag="t512", bufs=4)
                nc.tensor.matmul(pss[:], ones_col[:], sq[:],
                                 start=True, stop=True)
                ssb = btmp.tile([1, 512], F32, tag="ssb")
                nc.scalar.activation(ssb[:], pss[:], ACTF.Sqrt,
                                     bias=epst[:], scale=sscale)
                sdr = dscr.tile([1, 512], F32, tag="sdr")
                nc.sync.dma_start(sdr[:], ssb[:])
                sbc = btmp.tile([128, 512], F32, tag="sbc")
                nc.sync.dma_start(
                    sbc[:], bass.AP(tensor=sdr[:].tensor, offset=sdr[:].offset,
                                    ap=[[0, 128]] + sdr[:].ap[1:]))
                t1 = btmp.tile([128, 512], F32, tag="t1")
                nc.vector.tensor_mul(t1[:], src[:, sl], sbc[:])
                rot = btmp.tile([128, 512], F32, tag="rot")
                nc.vector.tensor_scalar_mul(rot[0:64, :], t1[64:128, :], -1.0)
                nc.vector.tensor_copy(rot[64:128, :], t1[0:64, :])
                qr = btmp.tile([128, 512], F32, tag="qr")
                nc.vector.tensor_mul(qr[:], t1[:], cos_sb[:, sl])
                nc.vector.tensor_mul(rot[:], rot[:], sin_sb[:, sl])
                nc.vector.tensor_add(qr[:], qr[:], rot[:])
                nc.scalar.copy(dsth[:, sl], qr[:])
                nc.vector.tensor_sub(dstl[:, sl], qr[:], dsth[:, sl])

        # ================= Phase C/D: attention + O proj =================
        for i in reversed(range(NMB)):
            nchunks = (i // 4 + 1) if is_causal else NCH
            attn16 = cpool.tile([128, HPC, 128], F16, tag="attn16")
            s_sbs, negms, lpartss, out_pss = [], [], [], []
            out_ps2_shared = [None]
            # pass 1 (both heads): scores (3x fp16 split matmuls), row maxes
            for h in range(HPC):
                qh_blk = qh16[:, h, i * 128:(i + 1) * 128]
                ql_blk = ql16[:, h, i * 128:(i + 1) * 128]
                s_sb = cpool.tile([128, NCH, 512], F32, tag="s_sb",
                                  name=f"s_sb_{i}_{h}")
                for ncj in range(nchunks):
                    ks = slice(ncj * 512, (ncj + 1) * 512)
                    ps_s = psum.tile([128, 512], F32, tag="t512", bufs=4,
                                     name=f"ps_s_{i}_{h}_{ncj}")
                    nc.tensor.matmul(ps_s[:], qh_blk, kh16[:, ks],
                                     start=True, stop=False)
                    nc.tensor.matmul(ps_s[:], qh_blk, kl16[:, ks],
                                     start=False, stop=False)
                    nc.tensor.matmul(ps_s[:], ql_blk, kh16[:, ks],
                                     start=False, stop=True)
                    if is_causal and ncj == i // 4:
                        nc.vector.tensor_add(s_sb[:, ncj, :], ps_s[:],
                                             cmask[:, i % 4, :])
                    elif not is_causal:
                        mload = cpool.tile([128, 512], F32, tag="mload",
                                           bufs=3, name=f"mload_{i}_{h}_{ncj}")
                        nc.sync.dma_start(
                            mload[:], mask_d[i * 128:(i + 1) * 128, ks])
                        nc.vector.tensor_add(s_sb[:, ncj, :], ps_s[:],
                                             mload[:])
                    else:
                        nc.scalar.copy(s_sb[:, ncj, :], ps_s[:])
                negm = cpool.tile([128, 1], F32, tag="negm",
                                  name=f"negm_{i}_{h}")
                nc.vector.reduce_max(negm[:], s_sb[:, 0:nchunks, :],
                                     axis=mybir.AxisListType.XY, negate=True)
                s_sbs.append(s_sb)
                negms.append(negm)
            # pass 2 (both heads): exp (fp16) -> PE transpose -> PV accumulate
            for h in range(HPC):
                s_sb, negm = s_sbs[h], negms[h]
                lparts = cpool.tile([128, NCH], F32, tag="lparts",
                                    name=f"lparts_{i}_{h}")
                if h == 0:
                    out_ps2 = psum.tile([128, HPC, 128], F32, tag="t128",
                                        bufs=3, name=f"out_ps2_{i}")
                    out_ps2_shared[0] = out_ps2
                out_ps = out_ps2_shared[0][:, h, :]
                last_nkb = i if is_causal else nchunks * 4 - 1
                for ncj in range(nchunks):
                    if ncj % 2 == 0:
                        w = min(2, nchunks - ncj)
                        p16w = cpool.tile([128, 2, 512], F16, tag="p16",
                                          name=f"p16_{i}_{h}_{ncj}")
                        nc.scalar.activation(
                            p16w[:, 0:w, :], s_sb[:, ncj:ncj + w, :],
                            ACTF.Exp, bias=negm[:], scale=1.0,
                            accum_out=lparts[:, ncj // 2:ncj // 2 + 1])
                    p16 = p16w[:, ncj % 2, :]
                    nb = min(4, last_nkb + 1 - ncj * 4)
                    ps_t4 = psum.tile([128, 4, 128], F16, tag="t128", bufs=3,
                                      name=f"ps_t4_{i}_{h}_{ncj}")
                    for b in range(nb):
                        nc.tensor.transpose(
                            ps_t4[:, b, :], p16[:, b * 128:(b + 1) * 128],
                            ident16[:])
                    pt_sb = cpool.tile([128, 4, 128], F16, tag="pt_sb",
                                       bufs=4, name=f"pt_sb_{i}_{h}_{ncj}")
                    if ncj % 2 == 0:
                        nc.vector.tensor_copy(pt_sb[:, 0:nb, :],
                                              ps_t4[:, 0:nb, :])
                    else:
                        nc.scalar.copy(pt_sb[:, 0:nb, :], ps_t4[:, 0:nb, :])
                    for b in range(nb):
                        nkb = ncj * 4 + b
                        nc.tensor.matmul(out_ps, pt_sb[:, b, :],
                                         v_sb[:, nkb],
                                         start=(nkb == 0),
                                         stop=(nkb == last_nkb))
                lpartss.append(lparts)
                out_pss.append(out_ps)
            for h in range(HPC):
                lsum = cpool.tile([128, 1], F32, tag="lsum",
                                  name=f"lsum_{i}_{h}")
                nc.vector.reduce_sum(lsum[:], lpartss[h][:, 0:(nchunks + 1) // 2],
                                     axis=AX)
                linv = cpool.tile([128, 1], F32, tag="linv",
                                  name=f"linv_{i}_{h}")
                nc.vector.reciprocal(linv[:], lsum[:])
                at = cpool.tile([128, 128], F16, tag="at", name=f"at_{i}_{h}")
                nc.vector.tensor_scalar_mul(at[:], out_pss[h], linv[:])
                pat = psum.tile([128, 128], F16, tag="t128", bufs=3,
                                name=f"pat_{i}_{h}")
                nc.tensor.transpose(pat[:], at[:], ident16[:])
                nc.vector.tensor_copy(attn16[:, h], pat[:])
            # O proj partial: out[m, n] += attnT[f, m].T @ wo[f, n]
            for nh_ in range(4):
                ns = slice(nh_ * 512, (nh_ + 1) * 512)
                po = psum.tile([128, 512], F32, tag="pod", bufs=1,
                               name=f"po_{i}_{nh_}")
                nc.tensor.matmul(po[:], attn16[:, 0], woh_sb[:, 0, ns],
                                 start=True, stop=False)
                nc.tensor.matmul(po[:], attn16[:, 1], woh_sb[:, 1, ns],
                                 start=False, stop=True)
                ob = dpool.tile([128, 512], F32, tag="ob",
                                name=f"ob_{i}_{nh_}")
                if nh_ % 2 == 0:
                    nc.vector.tensor_copy(ob[:], po[:])
                else:
                    nc.scalar.copy(ob[:], po[:])
                nc.sync.dma_start(out_d[i * 128:(i + 1) * 128, ns], ob[:])

    nc.compile()
    return nc


def _split16(a):
    hi = a.astype(np.float16)
    lo = (a - hi.astype(np.float32)).astype(np.float16)
    return hi, lo


def kernel(**inputs):
    x = np.asarray(inputs["x"], np.float32)
    cos = np.asarray(inputs["cos"], np.float32)
    sin = np.asarray(inputs["sin"], np.float32)
    am = np.asarray(inputs["attention_mask"]).reshape(S, S).astype(bool)
    wq = np.asarray(inputs["wq"], np.float32)
    bq = np.asarray(inputs["bq"], np.float32)
    wk = np.asarray(inputs["wk"], np.float32)
    bk = np.asarray(inputs["bk"], np.float32)
    wv = np.asarray(inputs["wv"], np.float32)
    bv = np.asarray(inputs["bv"], np.float32)
    wo = np.asarray(inputs["wo"], np.float32)
    bo = np.asarray(inputs["bo"], np.float32)
    qn = np.asarray(inputs["q_norm_w"], np.float32)
    kn = np.asarray(inputs["k_norm_w"], np.float32)

    assert x.shape == (1, S, H)
    is_causal = bool(
        (am == np.triu(np.ones((S, S), dtype=bool), k=1)).all())

    key = is_causal
    if key not in _prog_cache:
        _prog_cache[key] = _build(is_causal)
    nc = _prog_cache[key]

    xT = np.ascontiguousarray(x[0].T)
    xth, xtl = _split16(xT)
    cosT = cos.T
    sinT = sin.T
    rolled_q = np.roll(qn, -64)     # rot(q*qn)[i] = rot(q)[i] * qn[(i+64)%128]
    rolled_k = np.roll(kn, -64)
    cosq = np.ascontiguousarray(cosT * qn[:, None])
    sinq = np.ascontiguousarray(sinT * rolled_q[:, None])
    cosk = np.ascontiguousarray(cosT * kn[:, None])
    sink = np.ascontiguousarray(sinT * rolled_k[:, None])
    if not is_causal:
        maskadd = np.where(am, np.float32(NEG), np.float32(0.0))

    in_maps = []
    for c in range(NCORES):
        fq = slice(c * FQ, (c + 1) * FQ)
        g = c // 2
        fk = slice(g * HD, (g + 1) * HD)
        wqh, wql = _split16(wq[:, fq])
        wkh, wkl = _split16(wk[:, fk])
        m = dict(
            xth=xth, xtl=xtl,
            wqh=np.ascontiguousarray(wqh), wql=np.ascontiguousarray(wql),
            wkh=np.ascontiguousarray(wkh), wkl=np.ascontiguousarray(wkl),
            wvh=np.ascontiguousarray(wv[:, fk].astype(np.float16)),
            woh=np.ascontiguousarray(wo[fq, :].astype(np.float16)),
            cosq=cosq, sinq=sinq, cosk=cosk, sink=sink,
            bqt=np.ascontiguousarray(bq[fq].reshape(HPC, HD).T),
            bkt=np.ascontiguousarray(bk[fk].reshape(1, HD).T),
            bvt=np.ascontiguousarray(bv[fk].reshape(1, HD).T),
        )
        if not is_causal:
            m["maskadd"] = maskadd
        in_maps.append(m)

    res = bass_utils.run_bass_kernel_spmd(nc, in_maps,
                                          core_ids=list(range(NCORES)))
    acc = np.zeros((S, H), np.float64)
    for c in range(NCORES):
        acc += res.results[c]["out"]
    out = (acc + bo[None, :]).astype(np.float32)
    return out.reshape(1, S, H)


# revision 41
# speedup vs baseline: 27145.6832x; 1.0438x over previous
"""Trainium2 Bass kernel for nn_Attention_89902255440825.

Single-layer attention block: QKV proj + per-head RMS("mult" variant) +
RoPE + GQA causal attention with softmax(scores * sqrt(HD)) + O proj.

Sharding (8 NeuronCores, tensor-parallel over heads):
  core c: q heads {2c, 2c+1}  (wq cols 256c:256c+256)
          kv head c//2        (wk/wv cols 128*(c//2):...)
          wo rows 256c:256c+256  -> partial [S,H] outputs, summed on host.

Precision strategy (measured on HW):
  - q/k path (projections + scores) uses fp16 hi/lo split: 3 fp16 matmuls
    give ~fp32 accuracy at 3 cyc/row (native fp32 is 4 cyc/row).
    Required because softmax here is multiplied by sqrt(HD): logit std ~95,
    near-argmax attention, so logits need ~1e-5 relative accuracy.
  - v / PV / O-proj use single-pass fp16 (error ~1e-3, benign).

Layouts on device (per core):
  xT (hi/lo fp16)      [H=2048, S=2048]   host-pretransposed
  qT/kT feature-major  [d=128, m=2048]    d on partitions (scores lhsT/rhs)
  v    token-major     [m, d] tiles       (PV rhs)
  attn_outT via PE transpose -> O proj lhsT
"""
import numpy as np
from contextlib import ExitStack

import concourse.bass as bass
import concourse.tile as tile
from concourse import bacc, mybir, bass_utils
from concourse.masks import make_identity

S = 2048
H = 2048
HD = 128
NH = 16
NKV = 4
NCORES = 8
HPC = NH // NCORES          # q heads per core = 2
FQ = HPC * HD               # q features per core = 256
EPS = 1e-6
SM_SCALE = float(np.sqrt(HD))   # reference: softmax(scores / HD**-0.5)
NEG = -1e30
F32 = mybir.dt.float32
F16 = mybir.dt.float16
AX = mybir.AxisListType.X
OP = mybir.AluOpType
ACTF = mybir.ActivationFunctionType

_prog_cache = {}


def _build(is_causal: bool):
    nc = bacc.Bacc("TRN2", target_bir_lowering=False, debug=False,
                   num_devices=NCORES)

    def din(name, shape, dt=F16):
        return nc.dram_tensor(name, shape, dt, kind="ExternalInput").ap()

    xth_d = din("xth", [H, S])
    xtl_d = din("xtl", [H, S])
    wqh_d = din("wqh", [H, FQ])
    wql_d = din("wql", [H, FQ])
    wkh_d = din("wkh", [H, HD])
    wkl_d = din("wkl", [H, HD])
    wvh_d = din("wvh", [H, HD])
    woh_d = din("woh", [FQ, H])
    cosq_d = din("cosq", [HD, S], F32)
    sinq_d = din("sinq", [HD, S], F32)
    cosk_d = din("cosk", [HD, S], F32)
    sink_d = din("sink", [HD, S], F32)
    bqt_d = din("bqt", [HD, HPC], F32)
    bkt_d = din("bkt", [HD, 1], F32)
    bvt_d = din("bvt", [HD, 1], F32)
    if not is_causal:
        mask_d = din("maskadd", [S, S], F32)
    out_d = nc.dram_tensor("out", [S, H], F32, kind="ExternalOutput").ap()

    NKB = H // 128            # 16 contraction k-blocks
    NMB = S // 128            # 16 token blocks
    NCH = S // 512            # 4 512-chunks

    with tile.TileContext(nc) as tc, ExitStack() as ctx:
        const = ctx.enter_context(tc.tile_pool(name="const", bufs=1))
        wpool = ctx.enter_context(tc.tile_pool(name="wpool", bufs=1))
        big = ctx.enter_context(tc.tile_pool(name="big", bufs=1))
        xpool = ctx.enter_context(tc.tile_pool(name="xpool", bufs=3))
        btmp = ctx.enter_context(tc.tile_pool(name="btmp", bufs=2))
        cpool = ctx.enter_context(tc.tile_pool(name="cpool", bufs=2))
        dpool = ctx.enter_context(tc.tile_pool(name="dpool", bufs=3))
        psum = ctx.enter_context(tc.tile_pool(name="psum", bufs=1, space="PSUM"))
        dscr = ctx.enter_context(tc.tile_pool(name="dscr", bufs=3, space="DRAM"))

        # ---- constants ----
        ident16 = const.tile([128, 128], F16)
        make_identity(nc, ident16[:])
        ones_col = const.tile([128, 1], F32)
        nc.vector.memset(ones_col[:], 1.0)
        eps_q = const.tile([1, 1], F32)   # q: 11.31*sqrt(x/128+eps) = sqrt(x+128*eps)
        nc.vector.memset(eps_q[:], EPS * HD)
        eps_k = const.tile([1, 1], F32)
        nc.vector.memset(eps_k[:], EPS)
        if is_causal:
            cmask = const.tile([128, 4, 512], F32)
            for r in range(4):
                nc.vector.memset(cmask[:, r, :], 0.0)
                nc.gpsimd.affine_select(
                    out=cmask[:, r, :], in_=cmask[:, r, :],
                    compare_op=OP.is_ge, fill=NEG,
                    base=128 * r, channel_multiplier=1, pattern=[[-1, 512]],
                )

        # ---- weights / small inputs to SBUF ----
        def wtiles(dram, f, nm):
            t = wpool.tile([128, NKB, f], F16, name=nm, tag=nm)
            nc.sync.dma_start(t[:], dram.rearrange("(t p) f -> p t f", p=128))
            return t

        wqh_sb = wtiles(wqh_d, FQ, "wqh_sb")
        wql_sb = wtiles(wql_d, FQ, "wql_sb")
        wkh_sb = wtiles(wkh_d, HD, "wkh_sb")
        wkl_sb = wtiles(wkl_d, HD, "wkl_sb")
        wvh_sb = wtiles(wvh_d, HD, "wvh_sb")
        woh_sb = wpool.tile([128, HPC, H], F16)
        cosq_sb = wpool.tile([HD, S], F32)
        sinq_sb = wpool.tile([HD, S], F32)
        cosk_sb = wpool.tile([HD, S], F32)
        sink_sb = wpool.tile([HD, S], F32)
        bqt_sb = wpool.tile([HD, HPC], F32)
        nc.sync.dma_start(bqt_sb[:], bqt_d)
        bkt_sb = wpool.tile([HD, 1], F32)
        nc.sync.dma_start(bkt_sb[:], bkt_d)
        bvt_sb = wpool.tile([HD, 1], F32)
        nc.sync.dma_start(bvt_sb[:], bvt_d)

        # ---- persistent activations ----
        vt16 = big.tile([128, S], F16)            # v feature-major fp16
        v_sb = big.tile([128, NMB, 128], F16)     # v token-major fp16
        qh16 = big.tile([128, HPC, S], F16)       # roped+scaled q hi/lo
        ql16 = big.tile([128, HPC, S], F16)
        kh16 = big.tile([128, S], F16)
        kl16 = big.tile([128, S], F16)

        # ================= Phase A: QKV projections =================
        # feature-major: psum[f 128, m 512] += w[kb,f].T @ xT[kb, mquarter]
        for mq_ in range(4):
            ms = slice(mq_ * 512, (mq_ + 1) * 512)
            pq = [psum.tile([128, 512], F32, tag="t512", bufs=4, name=f"pq{fb}_{mq_}")
                  for fb in range(HPC)]
            pk = psum.tile([128, 512], F32, tag="t512", bufs=4)
            pv = psum.tile([128, 512], F32, tag="t512", bufs=4)
            for kb in range(NKB):
                xh_t = xpool.tile([128, 512], F16, tag="xh")
                nc.sync.dma_start(xh_t[:], xth_d[kb * 128:(kb + 1) * 128, ms])
                xl_t = xpool.tile([128, 512], F16, tag="xl")
                nc.sync.dma_start(xl_t[:], xtl_d[kb * 128:(kb + 1) * 128, ms])
                st = kb == 0
                sp = kb == NKB - 1
                for fb in range(HPC):
                    fsl = slice(fb * 128, (fb + 1) * 128)
                    nc.tensor.matmul(pq[fb][:], wqh_sb[:, kb, fsl], xh_t[:],
                                     start=st, stop=False)
                    nc.tensor.matmul(pq[fb][:], wqh_sb[:, kb, fsl], xl_t[:],
                                     start=False, stop=False)
                    nc.tensor.matmul(pq[fb][:], wql_sb[:, kb, fsl], xh_t[:],
                                     start=False, stop=sp)
                nc.tensor.matmul(pk[:], wkh_sb[:, kb, :], xh_t[:],
                                 start=st, stop=False)
                nc.tensor.matmul(pk[:], wkh_sb[:, kb, :], xl_t[:],
                                 start=False, stop=False)
                nc.tensor.matmul(pk[:], wkl_sb[:, kb, :], xh_t[:],
                                 start=False, stop=sp)
                nc.tensor.matmul(pv[:], wvh_sb[:, kb, :], xh_t[:],
                                 start=st, stop=sp)
            qt_q = btmp.tile([128, HPC, 512], F32, tag="qt_q",
                             name=f"qt_q_{mq_}")
            kt_q = btmp.tile([128, 512], F32, tag="kt_q", name=f"kt_q_{mq_}")
            for fb in range(HPC):
                nc.vector.tensor_scalar_add(qt_q[:, fb, :], pq[fb][:],
                                            bqt_sb[:, fb:fb + 1])
            nc.vector.tensor_scalar_add(kt_q[:], pk[:], bkt_sb[:])
            nc.vector.tensor_scalar_add(vt16[:, ms], pv[:], bvt_sb[:])

            if mq_ == 0:
                # loads needed from phase B onward; emitted after quarter 0's
                # matmuls so A's x-tile DMAs own the lanes at startup
                nc.sync.dma_start(cosk_sb[:], cosk_d)
                nc.sync.dma_start(sink_sb[:], sink_d)
                nc.sync.dma_start(cosq_sb[:], cosq_d)
                nc.sync.dma_start(sinq_sb[:], sinq_d)
                nc.sync.dma_start(
                    woh_sb[:], woh_d.rearrange("(t p) f -> p t f", p=128))

            # ---- phase B fused per quarter: RMS + RoPE + fp16 split ----
            specs = [
                (kt_q[:], kh16[:], kl16[:], eps_k, 1.0 / HD, cosk_sb, sink_sb),
                (qt_q[:, 0], qh16[:, 0], ql16[:, 0], eps_q, 1.0,
                 cosq_sb, sinq_sb),
                (qt_q[:, 1], qh16[:, 1], ql16[:, 1], eps_q, 1.0,
                 cosq_sb, sinq_sb),
            ]
            for bsrc, dsth, dstl, epst, sscale, cos_sb, sin_sb in specs:
                sq = btmp.tile([128, 512], F32, tag="sq")
                nc.scalar.activation(sq[:], bsrc, ACTF.Square)
                pss = psum.tile([1, 512], F32, tag="t512", bufs=4)
                nc.tensor.matmul(pss[:], ones_col[:], sq[:],
                                 start=True, stop=True)
                ssb = btmp.tile([1, 512], F32, tag="ssb")
                nc.scalar.activation(ssb[:], pss[:], ACTF.Sqrt,
                                     bias=epst[:], scale=sscale)
                sdr = dscr.tile([1, 512], F32, tag="sdr")
                nc.sync.dma_start(sdr[:], ssb[:])
                sbc = btmp.tile([128, 512], F32, tag="sbc")
                nc.sync.dma_start(
                    sbc[:], bass.AP(tensor=sdr[:].tensor, offset=sdr[:].offset,
                                    ap=[[0, 128]] + sdr[:].ap[1:]))
                t1 = btmp.tile([128, 512], F32, tag="t1")
                nc.vector.tensor_mul(t1[:], bsrc, sbc[:])
                rot = btmp.tile([128, 512], F32, tag="rot")
                nc.vector.tensor_scalar_mul(rot[0:64, :], t1[64:128, :], -1.0)
                nc.vector.tensor_copy(rot[64:128, :], t1[0:64, :])
                qr = btmp.tile([128, 512], F32, tag="qr")
                nc.vector.tensor_mul(qr[:], t1[:], cos_sb[:, ms])
                nc.vector.tensor_mul(rot[:], rot[:], sin_sb[:, ms])
                nc.vector.tensor_add(qr[:], qr[:], rot[:])
                nc.scalar.copy(dsth[:, ms], qr[:])
                nc.vector.tensor_sub(dstl[:, ms], qr[:], dsth[:, ms])

        # v: feature-major -> token-major via PE transpose
        for mb in range(NMB):
            pvt = psum.tile([128, 128], F16, tag="t128", bufs=3)
            nc.tensor.transpose(pvt[:], vt16[:, mb * 128:(mb + 1) * 128],
                                ident16[:])
            nc.vector.tensor_copy(v_sb[:, mb], pvt[:])

        # ================= Phase C/D: attention + O proj =================
        for i in reversed(range(NMB)):
            nchunks = (i // 4 + 1) if is_causal else NCH
            attn16 = cpool.tile([128, HPC, 128], F16, tag="attn16")
            s_sbs, negms, lpartss, out_pss = [], [], [], []
            out_ps2_shared = [None]
            # pass 1 (both heads): scores (3x fp16 split matmuls), row maxes
            for h in range(HPC):
                qh_blk = qh16[:, h, i * 128:(i + 1) * 128]
                ql_blk = ql16[:, h, i * 128:(i + 1) * 128]
                s_sb = cpool.tile([128, NCH, 512], F32, tag="s_sb",
                                  bufs=4, name=f"s_sb_{i}_{h}")
                for ncj in range(nchunks):
                    ks = slice(ncj * 512, (ncj + 1) * 512)
                    ps_s = psum.tile([128, 512], F32, tag="t512", bufs=4,
                                     name=f"ps_s_{i}_{h}_{ncj}")
                    nc.tensor.matmul(ps_s[:], qh_blk, kh16[:, ks],
                                     start=True, stop=False)
                    nc.tensor.matmul(ps_s[:], qh_blk, kl16[:, ks],
                                     start=False, stop=False)
                    nc.tensor.matmul(ps_s[:], ql_blk, kh16[:, ks],
                                     start=False, stop=True)
                    if is_causal and ncj == i // 4:
                        nc.vector.tensor_add(s_sb[:, ncj, :], ps_s[:],
                                             cmask[:, i % 4, :])
                    elif not is_causal:
                        mload = cpool.tile([128, 512], F32, tag="mload",
                                           bufs=3, name=f"mload_{i}_{h}_{ncj}")
                        nc.sync.dma_start(
                            mload[:], mask_d[i * 128:(i + 1) * 128, ks])
                        nc.vector.tensor_add(s_sb[:, ncj, :], ps_s[:],
                                             mload[:])
                    else:
                        nc.scalar.copy(s_sb[:, ncj, :], ps_s[:])
                negm = cpool.tile([128, 1], F32, tag="negm",
                                  name=f"negm_{i}_{h}")
                nc.vector.reduce_max(negm[:], s_sb[:, 0:nchunks, :],
                                     axis=mybir.AxisListType.XY, negate=True)
                s_sbs.append(s_sb)
                negms.append(negm)
            # pass 2 (both heads): exp (fp16) -> PE transpose -> PV accumulate
            for h in range(HPC):
                s_sb, negm = s_sbs[h], negms[h]
                lparts = cpool.tile([128, NCH], F32, tag="lparts",
                                    name=f"lparts_{i}_{h}")
                if h == 0:
                    out_ps2 = psum.tile([128, HPC, 128], F32, tag="t128",
                                        bufs=3, name=f"out_ps2_{i}")
                    out_ps2_shared[0] = out_ps2
                out_ps = out_ps2_shared[0][:, h, :]
                last_nkb = i if is_causal else nchunks * 4 - 1
                for ncj in range(nchunks):
                    if ncj % 2 == 0:
                        w = min(2, nchunks - ncj)
                        p16w = cpool.tile([128, 2, 512], F16, tag="p16", bufs=3,
                                          name=f"p16_{i}_{h}_{ncj}")
                        nc.scalar.activation(
                            p16w[:, 0:w, :], s_sb[:, ncj:ncj + w, :],
                            ACTF.Exp, bias=negm[:], scale=1.0,
                            accum_out=lparts[:, ncj // 2:ncj // 2 + 1])
                    p16 = p16w[:, ncj % 2, :]
                    nb = min(4, last_nkb + 1 - ncj * 4)
                    ps_t4 = psum.tile([128, 4, 128], F16, tag="t128", bufs=3,
                                      name=f"ps_t4_{i}_{h}_{ncj}")
                    for b in range(nb):
                        nc.tensor.transpose(
                            ps_t4[:, b, :], p16[:, b * 128:(b + 1) * 128],
                            ident16[:])
                    pt_sb = cpool.tile([128, 4, 128], F16, tag="pt_sb",
                                       bufs=4, name=f"pt_sb_{i}_{h}_{ncj}")
                    if ncj % 2 == 0:
                        nc.vector.tensor_copy(pt_sb[:, 0:nb, :],
                                              ps_t4[:, 0:nb, :])
                    else:
                        nc.scalar.copy(pt_sb[:, 0:nb, :], ps_t4[:, 0:nb, :])
                    for b in range(nb):
                        nkb = ncj * 4 + b
                        nc.tensor.matmul(out_ps, pt_sb[:, b, :],
                                         v_sb[:, nkb],
                                         start=(nkb == 0),
                                         stop=(nkb == last_nkb))
                lpartss.append(lparts)
                out_pss.append(out_ps)
            for h in range(HPC):
                lsum = cpool.tile([128, 1], F32, tag="lsum",
                                  name=f"lsum_{i}_{h}")
                nc.vector.reduce_sum(lsum[:], lpartss[h][:, 0:(nchunks + 1) // 2],
                                     axis=AX)
                linv = cpool.tile([128, 1], F32, tag="linv",
                                  name=f"linv_{i}_{h}")
                nc.vector.reciprocal(linv[:], lsum[:])
                at = cpool.tile([128, 128], F16, tag="at", name=f"at_{i}_{h}")
                nc.vector.tensor_scalar_mul(at[:], out_pss[h], linv[:])
                pat = psum.tile([128, 128], F16, tag="t128", bufs=3,
                                name=f"pat_{i}_{h}")
                nc.tensor.transpose(pat[:], at[:], ident16[:])
                nc.vector.tensor_copy(attn16[:, h], pat[:])
            # O proj partial: out[m, n] += attnT[f, m].T @ wo[f, n]
            for nh_ in range(4):
                ns = slice(nh_ * 512, (nh_ + 1) * 512)
                po = psum.tile([128, 512], F32, tag="pod", bufs=1,
                               name=f"po_{i}_{nh_}")
                nc.tensor.matmul(po[:], attn16[:, 0], woh_sb[:, 0, ns],
                                 start=True, stop=False)
                nc.tensor.matmul(po[:], attn16[:, 1], woh_sb[:, 1, ns],
                                 start=False, stop=True)
                ob = dpool.tile([128, 512], F32, tag="ob",
                                name=f"ob_{i}_{nh_}")
                if nh_ % 2 == 0:
                    nc.vector.tensor_copy(ob[:], po[:])
                else:
                    nc.scalar.copy(ob[:], po[:])
                nc.sync.dma_start(out_d[i * 128:(i + 1) * 128, ns], ob[:])

    nc.compile()
    return nc


def _split16(a):
    hi = a.astype(np.float16)
    lo = (a - hi.astype(np.float32)).astype(np.float16)
    return hi, lo


def kernel(**inputs):
    x = np.asarray(inputs["x"], np.float32)
    cos = np.asarray(inputs["cos"], np.float32)
    sin = np.asarray(inputs["sin"], np.float32)
    am = np.asarray(inputs["attention_mask"]).reshape(S, S).astype(bool)
    wq = np.asarray(inputs["wq"], np.float32)
    bq = np.asarray(inputs["bq"], np.float32)
    wk = np.asarray(inputs["wk"], np.float32)
    bk = np.asarray(inputs["bk"], np.float32)
    wv = np.asarray(inputs["wv"], np.float32)
    bv = np.asarray(inputs["bv"], np.float32)
    wo = np.asarray(inputs["wo"], np.float32)
    bo = np.asarray(inputs["bo"], np.float32)
    qn = np.asarray(inputs["q_norm_w"], np.float32)
    kn = np.asarray(inputs["k_norm_w"], np.float32)

    assert x.shape == (1, S, H)
    is_causal = bool(
        (am == np.triu(np.ones((S, S), dtype=bool), k=1)).all())

    key = is_causal
    if key not in _prog_cache:
        _prog_cache[key] = _build(is_causal)
    nc = _prog_cache[key]

    xT = np.ascontiguousarray(x[0].T)
    xth, xtl = _split16(xT)
    cosT = cos.T
    sinT = sin.T
    rolled_q = np.roll(qn, -64)     # rot(q*qn)[i] = rot(q)[i] * qn[(i+64)%128]
    rolled_k = np.roll(kn, -64)
    cosq = np.ascontiguousarray(cosT * qn[:, None])
    sinq = np.ascontiguousarray(sinT * rolled_q[:, None])
    cosk = np.ascontiguousarray(cosT * kn[:, None])
    sink = np.ascontiguousarray(sinT * rolled_k[:, None])
    if not is_causal:
        maskadd = np.where(am, np.float32(NEG), np.float32(0.0))

    in_maps = []
    for c in range(NCORES):
        fq = slice(c * FQ, (c + 1) * FQ)
        g = c // 2
        fk = slice(g * HD, (g + 1) * HD)
        wqh, wql = _split16(wq[:, fq])
        wkh, wkl = _split16(wk[:, fk])
        m = dict(
            xth=xth, xtl=xtl,
            wqh=np.ascontiguousarray(wqh), wql=np.ascontiguousarray(wql),
            wkh=np.ascontiguousarray(wkh), wkl=np.ascontiguousarray(wkl),
            wvh=np.ascontiguousarray(wv[:, fk].astype(np.float16)),
            woh=np.ascontiguousarray(wo[fq, :].astype(np.float16)),
            cosq=cosq, sinq=sinq, cosk=cosk, sink=sink,
            bqt=np.ascontiguousarray(bq[fq].reshape(HPC, HD).T),
            bkt=np.ascontiguousarray(bk[fk].reshape(1, HD).T),
            bvt=np.ascontiguousarray(bv[fk].reshape(1, HD).T),
        )
        if not is_causal:
            m["maskadd"] = maskadd
        in_maps.append(m)

    res = bass_utils.run_bass_kernel_spmd(nc, in_maps,
                                          core_ids=list(range(NCORES)))
    acc = np.zeros((S, H), np.float64)
    for c in range(NCORES):
        acc += res.results[c]["out"]
    out = (acc + bo[None, :]).astype(np.float32)
    return out.reshape(1, S, H)
